# revision 1
# baseline (speedup 1.0000x reference)
"""Causal multi-head attention with relative position bias on 8 Trainium2
NeuronCores.

Problem (full shapes): x[2,2048,1024], rel_bias[16,2048,2048],
w_qkv[1024,3072], b_qkv[3072], w_out[1024,1024], b_out[1024].

Sharding: core = (batch, head-group): 2 batches x 4 head-groups of 4 heads.
Each core computes q/k/v projections for its 4 heads, causal attention with
rel-bias, and a partial output projection through its heads' rows of w_out.
Host sums the 4 partial outputs per batch (the tensor-parallel reduce) and
adds b_out.

Device kernel design notes:
- Scores are computed TRANSPOSED (scoresT[kj,qi] = k.q) so no on-chip
  transposes are needed anywhere: softmax reduction over keys becomes a
  matmul contraction, handled by appending a ones-column to V; the PV matmul
  directly produces the transposed attention output that the out-projection
  needs as its stationary operand.
- exp(score + bias) = exp(score) * exp(bias): host precomputes exp(rel_biasT)
  in bf16 with the causal mask baked in as exact zeros. ACT does a pure exp
  straight from PSUM; DVE multiplies two bf16 SBUF operands at 2x rate.
- All matmul operands are bf16 (PSUM accumulation is fp32); softmax
  denominators, reciprocals and the normalization are fp32.
- The per-query normalization 1/denom is broadcast across partitions with a
  stride-0 SBUF->SBUF DMA and applied to the small attention output, not the
  big probability matrix.
"""

import math
import sys
import types
from contextlib import ExitStack

import ml_dtypes
import numpy as np

B, S, D = 2, 2048, 1024
NH, HD = 16, 64
NCORES = 8
HPC = 4  # heads per core (2 pairs)

_BF16 = ml_dtypes.bfloat16


def _install_ntff_hook():
    """concourse.bass_utils imports antenv.axon_hooks for NTFF tracing under
    axon; this container's antenv lacks that module. Provide it, backed by
    the ctypes hook from trn_agent_boot (if present)."""
    if "antenv.axon_hooks" in sys.modules:
        return
    try:
        import antenv
    except ImportError:
        return
    mod = types.ModuleType("antenv.axon_hooks")
    mod._hook = None
    mod.set_axon_ntff_profile_hook = lambda h: setattr(mod, "_hook", h)
    mod.get_axon_ntff_profile_hook = lambda: mod._hook
    sys.modules["antenv.axon_hooks"] = mod
    antenv.axon_hooks = mod
    try:
        from trn_agent_boot.trn_boot import _ntff_profile_via_ctypes

        h = _ntff_profile_via_ctypes("/opt/axon/libaxon_pjrt.so")
        if h is not None:
            mod._hook = h
    except Exception:
        pass


KC = D // 128   # 8 contraction chunks for the projections
NS4 = S // 512  # 4 s-superblocks
NSC = S // 128  # 16 s-chunks


def _phase_load(ctx, tc, nc, d, has_bqk, has_bv, st):
    """DMA weights + xT into persistent SBUF tiles."""
    from concourse import mybir
    bf = mybir.dt.bfloat16

    xt_pool = ctx.enter_context(tc.tile_pool(name="xt", bufs=KC))
    wqk_pool = ctx.enter_context(tc.tile_pool(name="wqk", bufs=KC))
    wv_pool = ctx.enter_context(tc.tile_pool(name="wv", bufs=KC))
    wo_pool = ctx.enter_context(tc.tile_pool(name="wo", bufs=2))
    const_pool = ctx.enter_context(tc.tile_pool(name="consts", bufs=1))

    st.ones_row = const_pool.tile([1, 512], bf)
    nc.gpsimd.memset(st.ones_row[:], 1.0)
    # selection rows for the denominator broadcast: selA hits partitions
    # 0-63, selB partitions 64-127 (K=1 matmuls accumulate both)
    sel_f32 = const_pool.tile([1, 256], mybir.dt.float32)
    nc.gpsimd.memset(sel_f32[:], 0.0)
    nc.gpsimd.memset(sel_f32[0:1, 0:64], 1.0)
    nc.gpsimd.memset(sel_f32[0:1, 192:256], 1.0)
    st.sel_f32r = const_pool.tile([1, 256], mybir.dt.float32r)
    nc.vector.tensor_copy(st.sel_f32r[:], sel_f32[:])

    st.wqk_t, st.xt_t, st.wv_t = [], [], []
    for k in range(KC):
        w = wqk_pool.tile([128, 512], bf)
        nc.sync.dma_start(w[:], d.wqk[k * 128:(k + 1) * 128, :])
        st.wqk_t.append(w)
        xt = xt_pool.tile([128, S], bf)
        nc.sync.dma_start(xt[:], d.xT[k * 128:(k + 1) * 128, :])
        st.xt_t.append(xt)
    for k in range(KC):
        # wv is first consumed ~30us in; keep it out of the critical
        # DMA prefix that the first qk accumulation chain waits on
        wv = wv_pool.tile([128, 260], bf)
        nc.sync.dma_start(wv[:], d.wv[k * 128:(k + 1) * 128, :])
        st.wv_t.append(wv)
    st.wo_t = []
    for p in range(2):
        w = wo_pool.tile([128, D], bf)
        nc.sync.dma_start(w[:], d.wo[p])
        st.wo_t.append(w)
    if has_bqk:
        st.bqk_sb = []
        for m in range(4):
            t = const_pool.tile([1, 128], bf, name=f"bqk{m}", tag=f"bqk{m}")
            nc.sync.dma_start(t[:], d.bqk[m:m + 1, :])
            st.bqk_sb.append(t)
    if has_bv:
        st.bv_sb = const_pool.tile([1, 260], bf)
        nc.sync.dma_start(st.bv_sb[:], d.bv[:])


def _phase_proj(ctx, tc, nc, has_bqk, has_bv, st):
    """qkv projections.

    qkT[m][r, s]: m-chunks 0..3 = [q pair0 | k pair0 | q pair1 | k pair1];
    within a chunk rows 0-63 = first head of the pair, 64-127 = second.
    v_t[si]: [128, 260] bf16, 4 slots of 65 cols (64 v-cols + ones col).
    """
    from concourse import mybir
    bf = mybir.dt.bfloat16
    f32 = mybir.dt.float32

    qkT_pool = ctx.enter_context(tc.tile_pool(name="qkT", bufs=4))
    v_pool = ctx.enter_context(tc.tile_pool(name="vsb", bufs=NSC))
    st.qkT_t = [qkT_pool.tile([128, S], bf, name="qkT", tag="qkT") for _ in range(4)]
    st.v_t = [v_pool.tile([128, 260], bf, name="vsb", tag="vsb") for _ in range(NSC)]

    def emit_qk(qk_ps, m):
        for s4 in range(NS4):
            ps = qk_ps.tile([128, 512], f32, name="qkps", tag="qkps")
            for k in range(KC):
                nc.tensor.matmul(
                    ps[:],
                    st.wqk_t[k][:, m * 128:(m + 1) * 128],
                    st.xt_t[k][:, s4 * 512:(s4 + 1) * 512],
                    start=(k == 0),
                    stop=(k == KC - 1 and not has_bqk),
                )
            if has_bqk:
                nc.tensor.matmul(
                    ps[:], st.bqk_sb[m][:], st.ones_row[:, :],
                    start=False, stop=True,
                )
            nc.vector.tensor_copy(
                st.qkT_t[m][:, s4 * 512:(s4 + 1) * 512], ps[:])

    with tc.tile_pool(name="qk_ps", bufs=4, space="PSUM") as qk_ps, \
         tc.tile_pool(name="v_ps", bufs=3, space="PSUM") as v_ps:
        for m in range(4):
            emit_qk(qk_ps, m)
        for si in range(NSC):
            ps = v_ps.tile([128, 260], f32)
            for k in range(KC):
                nc.tensor.matmul(
                    ps[:],
                    st.xt_t[k][:, si * 128:(si + 1) * 128],
                    st.wv_t[k][:],
                    start=(k == 0),
                    stop=(k == KC - 1 and not has_bv),
                )
            if has_bv:
                nc.tensor.matmul(
                    ps[:], st.ones_row[0:1, 0:128], st.bv_sb[:],
                    start=False, stop=True,
                )
            nc.scalar.copy(st.v_t[si][:], ps[:])
            for h in range(HPC):
                nc.gpsimd.memset(st.v_t[si][:, 65 * h + 64:65 * h + 65], 1.0)


def _attn_window(tc, nc, d, st, pools, p, qi8):
    """One qi window of 1024 for head-pair p: scores^T -> exp -> *exp(relT)
    -> PV accumulate -> normalize into attnT."""
    from concourse import mybir
    bf = mybir.dt.bfloat16
    f32 = mybir.dt.float32
    EXP = mybir.ActivationFunctionType.Exp
    (sc_ps, pv_ps, erb_pool, esc_pool, prob_pool, rec_pool, bc_pool,
     dram_pool) = pools

    qT = st.qkT_t[2 * p]
    kT = st.qkT_t[2 * p + 1]
    w0 = qi8 * 1024
    w1 = w0 + 1024
    nkj = w1 // 128
    # pv accumulators: [head][q4] -> [65, 512]
    pv = [[pv_ps.tile([65, 512], f32, name="pv", tag="pv") for _ in range(2)]
          for _ in range(2)]

    for kj in range(nkj):
        qs = max(w0, (kj * 128) // 512 * 512)
        width = w1 - qs
        # both heads' score MMs adjacent: consecutive MMs hit alternating
        # PE row groups, letting LDWEIGHTS overlap the in-flight matmul
        sc = [sc_ps.tile([128, width], f32, name="sc", tag="sc")
              for _ in range(2)]
        for off in range(qs, w1, 512):
            for h in range(2):
                rows = slice(64 * h, 64 * h + 64)
                nc.tensor.matmul(
                    sc[h][:, off - qs:off - qs + 512],
                    kT[rows, kj * 128:(kj + 1) * 128],
                    qT[rows, off:off + 512],
                    start=True, stop=True,
                    tile_position=(64 * h, 0),
                )
        pr = [None, None]
        for h in range(2):
            hl = 2 * p + h  # local head index
            esc = esc_pool.tile([128, width], bf, name="esc", tag="esc")
            nc.scalar.activation(esc[:], sc[h][:], EXP)
            rb = erb_pool.tile([128, width], bf, name="erb", tag="erb")
            nc.sync.dma_start(
                rb[:], d.erb[hl, kj * 128:(kj + 1) * 128, qs:w1])
            pr[h] = prob_pool.tile([128, width], bf, name="prob", tag="prob")
            nc.vector.tensor_mul(pr[h][:], esc[:], rb[:])
        for h in range(2):
            hl = 2 * p + h
            for off in range(qs, w1, 512):
                q4 = (off - w0) // 512
                last_kj = 8 * qi8 + 4 * q4 + 3
                nc.tensor.matmul(
                    pv[h][q4][:],
                    st.v_t[kj][:, 65 * hl:65 * hl + 65],
                    pr[h][:, off - qs:off - qs + 512],
                    start=(kj == 0),
                    stop=(kj == last_kj),
                )
    # normalization for this qi window
    rec = [rec_pool.tile([1, 1024], f32, name="rec", tag="rec") for _ in range(2)]
    for h in range(2):
        for q4 in range(2):
            nc.vector.reciprocal(
                rec[h][0:1, q4 * 512:(q4 + 1) * 512],
                pv[h][q4][64:65, :])
    bc = bc_pool.tile([128, 1024], f32, name="bc", tag="bc")
    for h in range(2):
        # DMA cannot broadcast an SBUF source across partitions; bounce the
        # reciprocal row through DRAM, where a zero-stride read is legal.
        dbounce = dram_pool.tile([1, 1024], f32, name="recd", tag="recd")
        nc.sync.dma_start(dbounce[:], rec[h][:])
        nc.sync.dma_start(
            bc[64 * h:64 * h + 64, :],
            dbounce[0:1, :].partition_broadcast(64))
    for h in range(2):
        for q4 in range(2):
            nc.vector.tensor_mul(
                st.attnT_t[p][64 * h:64 * h + 64,
                              w0 + q4 * 512:w0 + (q4 + 1) * 512],
                pv[h][q4][0:64, :],
                bc[64 * h:64 * h + 64, q4 * 512:(q4 + 1) * 512])


def _phase_attn(ctx, tc, nc, d, st):
    from concourse import mybir
    bf = mybir.dt.bfloat16

    attnT_pool = ctx.enter_context(tc.tile_pool(name="attnT", bufs=2))
    st.attnT_t = [attnT_pool.tile([128, S], bf, name="attnT", tag="attnT") for _ in range(2)]

    with ExitStack() as cctx:
        pools = (
            cctx.enter_context(tc.tile_pool(name="sc_ps", bufs=2, space="PSUM")),
            cctx.enter_context(tc.tile_pool(name="pv_ps", bufs=4, space="PSUM")),
            cctx.enter_context(tc.tile_pool(name="erb", bufs=10)),
            cctx.enter_context(tc.tile_pool(name="esc", bufs=8)),
            cctx.enter_context(tc.tile_pool(name="prob", bufs=8)),
            cctx.enter_context(tc.tile_pool(name="rec", bufs=4)),
            cctx.enter_context(tc.tile_pool(name="bc", bufs=3)),
            cctx.enter_context(tc.tile_pool(name="recd", bufs=8, space="DRAM")),
        )
        for p in range(2):
            for qi8 in range(2):
                _attn_window(tc, nc, d, st, pools, p, qi8)


def _phase_out(ctx, tc, nc, d, st):
    from concourse import mybir
    f32 = mybir.dt.float32

    with tc.tile_pool(name="o_ps", bufs=2, space="PSUM") as o_ps, \
         tc.tile_pool(name="osb", bufs=4) as osb_pool:
        for si in range(NSC):
            for e2 in range(2):
                ps = o_ps.tile([128, 512], f32, name="ops", tag="ops")
                for p in range(2):
                    nc.tensor.matmul(
                        ps[:],
                        st.attnT_t[p][:, si * 128:(si + 1) * 128],
                        st.wo_t[p][:, e2 * 512:(e2 + 1) * 512],
                        start=(p == 0), stop=(p == 1),
                    )
                osb = osb_pool.tile([128, 512], f32, name="osb", tag="osb")
                if e2 == 0:
                    nc.vector.tensor_copy(osb[:], ps[:])
                else:
                    nc.scalar.copy(osb[:], ps[:])
                nc.sync.dma_start(
                    d.out[si * 128:(si + 1) * 128, e2 * 512:(e2 + 1) * 512],
                    osb[:])


_LDW_OPT_INSTALLED = False


def _enable_ldw_opt():
    """walrus ships with --enable-ldw-opt=false; flip it for this process
    (dedupes/hoists LDWEIGHTS). Gated by KERNEL_LDW_OPT=1."""
    global _LDW_OPT_INSTALLED
    if _LDW_OPT_INSTALLED:
        return
    _LDW_OPT_INSTALLED = True
    import os
    if os.environ.get("KERNEL_LDW_OPT", "0") != "1":
        return
    import concourse.bass_utils as bu
    orig = bu.run_command

    def patched(argv, **kwargs):
        argv = ["--enable-ldw-opt=true" if a == "--enable-ldw-opt=false" else a
                for a in argv]
        return orig(argv, **kwargs)

    bu.run_command = patched


def _build_program(has_bqk: bool, has_bv: bool):
    import concourse.tile as tile
    from concourse import bacc, mybir

    bf = mybir.dt.bfloat16
    f32 = mybir.dt.float32

    nc = bacc.Bacc("TRN2", target_bir_lowering=False, debug=False,
                   num_devices=NCORES)

    d = types.SimpleNamespace()
    d.xT = nc.dram_tensor("xT", [D, S], bf, kind="ExternalInput").ap()
    d.wqk = nc.dram_tensor("wqk", [D, 512], bf, kind="ExternalInput").ap()
    d.wv = nc.dram_tensor("wv", [D, 260], bf, kind="ExternalInput").ap()
    d.bqk = nc.dram_tensor("bqk", [4, 128], bf, kind="ExternalInput").ap()
    d.bv = nc.dram_tensor("bv", [1, 260], bf, kind="ExternalInput").ap()
    d.erb = nc.dram_tensor("erb", [HPC, S, S], bf, kind="ExternalInput").ap()
    d.wo = nc.dram_tensor("wo", [2, 128, D], bf, kind="ExternalInput").ap()
    d.out = nc.dram_tensor("out", [S, D], f32, kind="ExternalOutput").ap()

    st = types.SimpleNamespace()
    with tile.TileContext(nc) as tc:
        with ExitStack() as ctx:
            _phase_load(ctx, tc, nc, d, has_bqk, has_bv, st)
            _phase_proj(ctx, tc, nc, has_bqk, has_bv, st)
            _phase_attn(ctx, tc, nc, d, st)
            _phase_out(ctx, tc, nc, d, st)

    nc.compile()
    return nc


_PROGRAM_CACHE = {}


def _get_program(has_bqk, has_bv):
    key = (has_bqk, has_bv)
    if key not in _PROGRAM_CACHE:
        _PROGRAM_CACHE[key] = _build_program(has_bqk, has_bv)
    return _PROGRAM_CACHE[key]


_last_results = None  # BassKernelResults of the most recent run (for test.py)


def kernel(x, rel_bias, w_qkv, b_qkv, w_out, b_out, *, trace=False):
    global _last_results
    _install_ntff_hook()
    _enable_ldw_opt()
    from concourse.bass_utils import run_bass_kernel_spmd

    x = np.asarray(x, dtype=np.float32)
    rel_bias = np.asarray(rel_bias, dtype=np.float32)
    w_qkv = np.asarray(w_qkv, dtype=np.float32)
    b_qkv = np.asarray(b_qkv, dtype=np.float32)
    w_out = np.asarray(w_out, dtype=np.float32)
    b_out = np.asarray(b_out, dtype=np.float32)

    wq = w_qkv[:, 0:D]
    wk = w_qkv[:, D:2 * D]
    wv = w_qkv[:, 2 * D:3 * D]
    bq, bk, bv = b_qkv[0:D], b_qkv[D:2 * D], b_qkv[2 * D:3 * D]
    has_bqk = bool(np.any(bq)) or bool(np.any(bk))
    has_bv = bool(np.any(bv))

    nc = _get_program(has_bqk, has_bv)

    sc = 1.0 / math.sqrt(HD)  # folded into the q projection
    xT = [np.ascontiguousarray(x[b].T).astype(_BF16) for b in range(B)]
    tri = np.triu(np.ones((S, S), dtype=np.float32))  # [kj, qi]: qi >= kj

    in_maps = []
    for c in range(NCORES):
        b, hg = divmod(c, 4)
        hs = [4 * hg + i for i in range(HPC)]

        # wqk columns: [q_h0 | q_h1 | k_h0 | k_h1 | q_h2 | q_h3 | k_h2 | k_h3]
        cols = []
        bqk_rows = []
        for pair in range(2):
            h0, h1 = hs[2 * pair], hs[2 * pair + 1]
            cols += [wq[:, HD * h0:HD * (h0 + 1)] * sc,
                     wq[:, HD * h1:HD * (h1 + 1)] * sc]
            bqk_rows.append(np.concatenate(
                [bq[HD * h0:HD * (h0 + 1)], bq[HD * h1:HD * (h1 + 1)]]) * sc)
            cols += [wk[:, HD * h0:HD * (h0 + 1)],
                     wk[:, HD * h1:HD * (h1 + 1)]]
            bqk_rows.append(np.concatenate(
                [bk[HD * h0:HD * (h0 + 1)], bk[HD * h1:HD * (h1 + 1)]]))
        wqk_c = np.concatenate(cols, axis=1).astype(_BF16)
        bqk_c = np.stack(bqk_rows).astype(_BF16)

        wv_c = np.zeros((D, 260), dtype=np.float32)
        bv_c = np.zeros((1, 260), dtype=np.float32)
        for i, h in enumerate(hs):
            wv_c[:, 65 * i:65 * i + 64] = wv[:, HD * h:HD * (h + 1)]
            bv_c[0, 65 * i:65 * i + 64] = bv[HD * h:HD * (h + 1)]

        erb_c = np.empty((HPC, S, S), dtype=_BF16)
        for i, h in enumerate(hs):
            erb_c[i] = (np.exp(rel_bias[h].T) * tri).astype(_BF16)

        in_maps.append({
            "xT": xT[b],
            "wqk": wqk_c,
            "wv": wv_c.astype(_BF16),
            "bqk": bqk_c,
            "bv": bv_c.astype(_BF16),
            "erb": erb_c,
            "wo": np.ascontiguousarray(
                w_out[256 * hg:256 * (hg + 1)].reshape(2, 128, D)).astype(_BF16),
        })

    res = run_bass_kernel_spmd(nc, in_maps, list(range(NCORES)), trace=trace)
    _last_results = res

    out = np.zeros((B, S, D), dtype=np.float32)
    for c in range(NCORES):
        out[c // 4] += res.results[c]["out"]
    out += b_out
    return out



# revision 3
# speedup vs baseline: 1.1035x; 1.1035x over previous
"""Causal multi-head attention with relative position bias on 8 Trainium2
NeuronCores.

Problem (full shapes): x[2,2048,1024], rel_bias[16,2048,2048],
w_qkv[1024,3072], b_qkv[3072], w_out[1024,1024], b_out[1024].

Sharding: core = (batch, head-group): 2 batches x 4 head-groups of 4 heads.
Each core computes q/k/v projections for its 4 heads, causal attention with
rel-bias, and a partial output projection through its heads' rows of w_out.
Host sums the 4 partial outputs per batch (the tensor-parallel reduce) and
adds b_out.

Device kernel design notes:
- Scores are computed TRANSPOSED (scoresT[kj,qi] = k.q) so no on-chip
  transposes are needed anywhere: softmax reduction over keys becomes a
  matmul contraction, handled by appending a ones-column to V; the PV matmul
  directly produces the transposed attention output that the out-projection
  needs as its stationary operand.
- exp(score + bias) = exp(score) * exp(bias): host precomputes exp(rel_biasT)
  in bf16 with the causal mask baked in as exact zeros. ACT does a pure exp
  straight from PSUM; DVE multiplies two bf16 SBUF operands at 2x rate.
- All matmul operands are bf16 (PSUM accumulation is fp32); softmax
  denominators, reciprocals and the normalization are fp32.
- The per-query normalization 1/denom is broadcast across partitions with a
  stride-0 SBUF->SBUF DMA and applied to the small attention output, not the
  big probability matrix.
"""

import math
import sys
import types
from contextlib import ExitStack

import ml_dtypes
import numpy as np

B, S, D = 2, 2048, 1024
NH, HD = 16, 64
NCORES = 8
HPC = 4  # heads per core (2 pairs)

_BF16 = ml_dtypes.bfloat16


def _install_ntff_hook():
    """concourse.bass_utils imports antenv.axon_hooks for NTFF tracing under
    axon; this container's antenv lacks that module. Provide it, backed by
    the ctypes hook from trn_agent_boot (if present)."""
    if "antenv.axon_hooks" in sys.modules:
        return
    try:
        import antenv
    except ImportError:
        return
    mod = types.ModuleType("antenv.axon_hooks")
    mod._hook = None
    mod.set_axon_ntff_profile_hook = lambda h: setattr(mod, "_hook", h)
    mod.get_axon_ntff_profile_hook = lambda: mod._hook
    sys.modules["antenv.axon_hooks"] = mod
    antenv.axon_hooks = mod
    try:
        from trn_agent_boot.trn_boot import _ntff_profile_via_ctypes

        h = _ntff_profile_via_ctypes("/opt/axon/libaxon_pjrt.so")
        if h is not None:
            mod._hook = h
    except Exception:
        pass


KC = D // 128   # 8 contraction chunks for the projections
NS4 = S // 512  # 4 s-superblocks
NSC = S // 128  # 16 s-chunks


def _phase_load(ctx, tc, nc, d, has_bqk, has_bv, st):
    """DMA weights + xT into persistent SBUF tiles."""
    from concourse import mybir
    bf = mybir.dt.bfloat16

    xt_pool = ctx.enter_context(tc.tile_pool(name="xt", bufs=KC))
    wqk_pool = ctx.enter_context(tc.tile_pool(name="wqk", bufs=KC))
    wv_pool = ctx.enter_context(tc.tile_pool(name="wv", bufs=KC))
    wo_pool = ctx.enter_context(tc.tile_pool(name="wo", bufs=2))
    const_pool = ctx.enter_context(tc.tile_pool(name="consts", bufs=1))

    st.ones_row = const_pool.tile([1, 512], bf)
    nc.gpsimd.memset(st.ones_row[:], 1.0)
    # selection rows for the denominator broadcast: selA hits partitions
    # 0-63, selB partitions 64-127 (K=1 matmuls accumulate both)
    sel_f32 = const_pool.tile([1, 256], mybir.dt.float32)
    nc.gpsimd.memset(sel_f32[:], 0.0)
    nc.gpsimd.memset(sel_f32[0:1, 0:64], 1.0)
    nc.gpsimd.memset(sel_f32[0:1, 192:256], 1.0)
    st.sel_f32r = const_pool.tile([1, 256], mybir.dt.float32r)
    nc.vector.tensor_copy(st.sel_f32r[:], sel_f32[:])

    st.wqk_t, st.xt_t, st.wv_t = [], [], []
    for k in range(KC):
        w = wqk_pool.tile([128, 512], bf)
        nc.sync.dma_start(w[:], d.wqk[k * 128:(k + 1) * 128, :])
        st.wqk_t.append(w)
        xt = xt_pool.tile([128, S], bf)
        nc.sync.dma_start(xt[:], d.xT[k * 128:(k + 1) * 128, :])
        st.xt_t.append(xt)
    for k in range(KC):
        # wv is first consumed ~30us in; keep it out of the critical
        # DMA prefix that the first qk accumulation chain waits on
        wv = wv_pool.tile([128, 260], bf)
        nc.sync.dma_start(wv[:], d.wv[k * 128:(k + 1) * 128, :])
        st.wv_t.append(wv)
    st.wo_t = []
    for p in range(2):
        w = wo_pool.tile([128, D], bf)
        nc.sync.dma_start(w[:], d.wo[p])
        st.wo_t.append(w)
    if has_bqk:
        st.bqk_sb = []
        for m in range(4):
            t = const_pool.tile([1, 128], bf, name=f"bqk{m}", tag=f"bqk{m}")
            nc.sync.dma_start(t[:], d.bqk[m:m + 1, :])
            st.bqk_sb.append(t)
    if has_bv:
        st.bv_sb = const_pool.tile([1, 260], bf)
        nc.sync.dma_start(st.bv_sb[:], d.bv[:])


def _phase_proj(ctx, tc, nc, has_bqk, has_bv, st):
    """qkv projections.

    qkT[m][r, s]: m-chunks 0..3 = [q pair0 | k pair0 | q pair1 | k pair1];
    within a chunk rows 0-63 = first head of the pair, 64-127 = second.
    v_t[si]: [128, 260] bf16, 4 slots of 65 cols (64 v-cols + ones col).
    """
    from concourse import mybir
    bf = mybir.dt.bfloat16
    f32 = mybir.dt.float32

    qkT_pool = ctx.enter_context(tc.tile_pool(name="qkT", bufs=4))
    v_pool = ctx.enter_context(tc.tile_pool(name="vsb", bufs=NSC))
    st.qkT_t = [qkT_pool.tile([128, S], bf, name="qkT", tag="qkT") for _ in range(4)]
    st.v_t = [v_pool.tile([128, 260], bf, name="vsb", tag="vsb") for _ in range(NSC)]

    def emit_qk(qk_ps, m):
        for s4 in range(NS4):
            ps = qk_ps.tile([128, 512], f32, name="qkps", tag="qkps")
            for k in range(KC):
                nc.tensor.matmul(
                    ps[:],
                    st.wqk_t[k][:, m * 128:(m + 1) * 128],
                    st.xt_t[k][:, s4 * 512:(s4 + 1) * 512],
                    start=(k == 0),
                    stop=(k == KC - 1 and not has_bqk),
                )
            if has_bqk:
                nc.tensor.matmul(
                    ps[:], st.bqk_sb[m][:], st.ones_row[:, :],
                    start=False, stop=True,
                )
            nc.vector.tensor_copy(
                st.qkT_t[m][:, s4 * 512:(s4 + 1) * 512], ps[:])

    with tc.tile_pool(name="qk_ps", bufs=4, space="PSUM") as qk_ps, \
         tc.tile_pool(name="v_ps", bufs=3, space="PSUM") as v_ps:
        for m in range(4):
            emit_qk(qk_ps, m)
        for si in range(NSC):
            ps = v_ps.tile([128, 260], f32)
            for k in range(KC):
                nc.tensor.matmul(
                    ps[:],
                    st.xt_t[k][:, si * 128:(si + 1) * 128],
                    st.wv_t[k][:],
                    start=(k == 0),
                    stop=(k == KC - 1 and not has_bv),
                )
            if has_bv:
                nc.tensor.matmul(
                    ps[:], st.ones_row[0:1, 0:128], st.bv_sb[:],
                    start=False, stop=True,
                )
            nc.scalar.copy(st.v_t[si][:], ps[:])
            for h in range(HPC):
                nc.gpsimd.memset(st.v_t[si][:, 65 * h + 64:65 * h + 65], 1.0)


def _attn_window(tc, nc, d, st, pools, p, qi8):
    """One qi window of 1024 for head-pair p: scores^T -> exp -> *exp(relT)
    -> PV accumulate -> normalize into attnT."""
    from concourse import mybir
    bf = mybir.dt.bfloat16
    f32 = mybir.dt.float32
    EXP = mybir.ActivationFunctionType.Exp
    (sc_ps, pv_ps, erb_pool, esc_pool, prob_pool, rec_pool, bc_pool,
     dram_pool) = pools

    qT = st.qkT_t[2 * p]
    kT = st.qkT_t[2 * p + 1]
    w0 = qi8 * 1024
    w1 = w0 + 1024
    nkj = w1 // 128
    # pv accumulators: [head][q4] -> [65, 512]
    pv = [[pv_ps.tile([65, 512], f32, name="pv", tag="pv") for _ in range(2)]
          for _ in range(2)]

    for kj in range(nkj):
        qs = max(w0, (kj * 128) // 512 * 512)
        width = w1 - qs
        # both heads' score MMs adjacent: consecutive MMs hit alternating
        # PE row groups, letting LDWEIGHTS overlap the in-flight matmul
        sc = [sc_ps.tile([128, width], f32, name="sc", tag="sc")
              for _ in range(2)]
        for off in range(qs, w1, 512):
            for h in range(2):
                rows = slice(64 * h, 64 * h + 64)
                nc.tensor.matmul(
                    sc[h][:, off - qs:off - qs + 512],
                    kT[rows, kj * 128:(kj + 1) * 128],
                    qT[rows, off:off + 512],
                    start=True, stop=True,
                    tile_position=(64 * h, 0),
                )
        pr = [None, None]
        for h in range(2):
            hl = 2 * p + h  # local head index
            esc = esc_pool.tile([128, width], bf, name="esc", tag="esc")
            nc.scalar.activation(esc[:], sc[h][:], EXP)
            rb = erb_pool.tile([128, width], bf, name="erb", tag="erb")
            nc.sync.dma_start(
                rb[:], d.erb[hl, kj * 128:(kj + 1) * 128, qs:w1])
            pr[h] = prob_pool.tile([128, width], bf, name="prob", tag="prob")
            nc.vector.tensor_mul(pr[h][:], esc[:], rb[:])
        for h in range(2):
            hl = 2 * p + h
            for off in range(qs, w1, 512):
                q4 = (off - w0) // 512
                last_kj = 8 * qi8 + 4 * q4 + 3
                nc.tensor.matmul(
                    pv[h][q4][:],
                    st.v_t[kj][:, 65 * hl:65 * hl + 65],
                    pr[h][:, off - qs:off - qs + 512],
                    start=(kj == 0),
                    stop=(kj == last_kj),
                )
    # normalization for this qi window
    rec = [rec_pool.tile([1, 1024], f32, name="rec", tag="rec") for _ in range(2)]
    for h in range(2):
        # the approx reciprocal's bit-trick seed needs IEEE fp32 layout, so
        # the denominators must leave PSUM (accumulator format) first
        den = rec_pool.tile([1, 1024], f32, name="den", tag="den")
        for q4 in range(2):
            nc.vector.tensor_copy(
                den[0:1, q4 * 512:(q4 + 1) * 512],
                pv[h][q4][64:65, :])
        # denominators are sums of positives in [~1, ~2e3]; the ~51-ULP
        # approximation is far below the bf16 noise floor already in play
        nc.vector.reciprocal_approx_fast(out=rec[h][:], in_=den[:])
    bc = bc_pool.tile([128, 1024], f32, name="bc", tag="bc")
    for h in range(2):
        # DMA cannot broadcast an SBUF source across partitions; bounce the
        # reciprocal row through DRAM, where a zero-stride read is legal.
        dbounce = dram_pool.tile([1, 1024], f32, name="recd", tag="recd")
        nc.sync.dma_start(dbounce[:], rec[h][:])
        nc.sync.dma_start(
            bc[64 * h:64 * h + 64, :],
            dbounce[0:1, :].partition_broadcast(64))
    for h in range(2):
        for q4 in range(2):
            nc.vector.tensor_mul(
                st.attnT_t[p][64 * h:64 * h + 64,
                              w0 + q4 * 512:w0 + (q4 + 1) * 512],
                pv[h][q4][0:64, :],
                bc[64 * h:64 * h + 64, q4 * 512:(q4 + 1) * 512])


def _phase_attn(ctx, tc, nc, d, st):
    from concourse import mybir
    bf = mybir.dt.bfloat16

    attnT_pool = ctx.enter_context(tc.tile_pool(name="attnT", bufs=2))
    st.attnT_t = [attnT_pool.tile([128, S], bf, name="attnT", tag="attnT") for _ in range(2)]

    with ExitStack() as cctx:
        pools = (
            cctx.enter_context(tc.tile_pool(name="sc_ps", bufs=2, space="PSUM")),
            cctx.enter_context(tc.tile_pool(name="pv_ps", bufs=4, space="PSUM")),
            cctx.enter_context(tc.tile_pool(name="erb", bufs=10)),
            cctx.enter_context(tc.tile_pool(name="esc", bufs=8)),
            cctx.enter_context(tc.tile_pool(name="prob", bufs=8)),
            cctx.enter_context(tc.tile_pool(name="rec", bufs=4)),
            cctx.enter_context(tc.tile_pool(name="bc", bufs=3)),
            cctx.enter_context(tc.tile_pool(name="recd", bufs=8, space="DRAM")),
        )
        for p in range(2):
            for qi8 in range(2):
                _attn_window(tc, nc, d, st, pools, p, qi8)


def _phase_out(ctx, tc, nc, d, st):
    from concourse import mybir
    f32 = mybir.dt.float32

    with tc.tile_pool(name="o_ps", bufs=2, space="PSUM") as o_ps, \
         tc.tile_pool(name="osb", bufs=4) as osb_pool:
        for si in range(NSC):
            for e2 in range(2):
                ps = o_ps.tile([128, 512], f32, name="ops", tag="ops")
                for p in range(2):
                    nc.tensor.matmul(
                        ps[:],
                        st.attnT_t[p][:, si * 128:(si + 1) * 128],
                        st.wo_t[p][:, e2 * 512:(e2 + 1) * 512],
                        start=(p == 0), stop=(p == 1),
                    )
                osb = osb_pool.tile([128, 512], f32, name="osb", tag="osb")
                if e2 == 0:
                    nc.vector.tensor_copy(osb[:], ps[:])
                else:
                    nc.scalar.copy(osb[:], ps[:])
                nc.sync.dma_start(
                    d.out[si * 128:(si + 1) * 128, e2 * 512:(e2 + 1) * 512],
                    osb[:])


_LDW_OPT_INSTALLED = False


def _enable_ldw_opt():
    """walrus ships with --enable-ldw-opt=false; flip it for this process
    (dedupes/hoists LDWEIGHTS). Gated by KERNEL_LDW_OPT=1."""
    global _LDW_OPT_INSTALLED
    if _LDW_OPT_INSTALLED:
        return
    _LDW_OPT_INSTALLED = True
    import os
    if os.environ.get("KERNEL_LDW_OPT", "0") != "1":
        return
    import concourse.bass_utils as bu
    orig = bu.run_command

    def patched(argv, **kwargs):
        argv = ["--enable-ldw-opt=true" if a == "--enable-ldw-opt=false" else a
                for a in argv]
        return orig(argv, **kwargs)

    bu.run_command = patched


def _build_program(has_bqk: bool, has_bv: bool):
    import concourse.tile as tile
    from concourse import bacc, mybir

    bf = mybir.dt.bfloat16
    f32 = mybir.dt.float32

    nc = bacc.Bacc("TRN2", target_bir_lowering=False, debug=False,
                   num_devices=NCORES)

    d = types.SimpleNamespace()
    d.xT = nc.dram_tensor("xT", [D, S], bf, kind="ExternalInput").ap()
    d.wqk = nc.dram_tensor("wqk", [D, 512], bf, kind="ExternalInput").ap()
    d.wv = nc.dram_tensor("wv", [D, 260], bf, kind="ExternalInput").ap()
    d.bqk = nc.dram_tensor("bqk", [4, 128], bf, kind="ExternalInput").ap()
    d.bv = nc.dram_tensor("bv", [1, 260], bf, kind="ExternalInput").ap()
    d.erb = nc.dram_tensor("erb", [HPC, S, S], bf, kind="ExternalInput").ap()
    d.wo = nc.dram_tensor("wo", [2, 128, D], bf, kind="ExternalInput").ap()
    d.out = nc.dram_tensor("out", [S, D], f32, kind="ExternalOutput").ap()

    st = types.SimpleNamespace()
    with tile.TileContext(nc) as tc:
        with ExitStack() as ctx:
            _phase_load(ctx, tc, nc, d, has_bqk, has_bv, st)
            _phase_proj(ctx, tc, nc, has_bqk, has_bv, st)
            _phase_attn(ctx, tc, nc, d, st)
            _phase_out(ctx, tc, nc, d, st)

    nc.compile()
    return nc


_PROGRAM_CACHE = {}


def _get_program(has_bqk, has_bv):
    key = (has_bqk, has_bv)
    if key not in _PROGRAM_CACHE:
        _PROGRAM_CACHE[key] = _build_program(has_bqk, has_bv)
    return _PROGRAM_CACHE[key]


_last_results = None  # BassKernelResults of the most recent run (for test.py)


def kernel(x, rel_bias, w_qkv, b_qkv, w_out, b_out, *, trace=False):
    global _last_results
    _install_ntff_hook()
    _enable_ldw_opt()
    from concourse.bass_utils import run_bass_kernel_spmd

    x = np.asarray(x, dtype=np.float32)
    rel_bias = np.asarray(rel_bias, dtype=np.float32)
    w_qkv = np.asarray(w_qkv, dtype=np.float32)
    b_qkv = np.asarray(b_qkv, dtype=np.float32)
    w_out = np.asarray(w_out, dtype=np.float32)
    b_out = np.asarray(b_out, dtype=np.float32)

    wq = w_qkv[:, 0:D]
    wk = w_qkv[:, D:2 * D]
    wv = w_qkv[:, 2 * D:3 * D]
    bq, bk, bv = b_qkv[0:D], b_qkv[D:2 * D], b_qkv[2 * D:3 * D]
    has_bqk = bool(np.any(bq)) or bool(np.any(bk))
    has_bv = bool(np.any(bv))

    nc = _get_program(has_bqk, has_bv)

    sc = 1.0 / math.sqrt(HD)  # folded into the q projection
    xT = [np.ascontiguousarray(x[b].T).astype(_BF16) for b in range(B)]
    tri = np.triu(np.ones((S, S), dtype=np.float32))  # [kj, qi]: qi >= kj

    in_maps = []
    for c in range(NCORES):
        b, hg = divmod(c, 4)
        hs = [4 * hg + i for i in range(HPC)]

        # wqk columns: [q_h0 | q_h1 | k_h0 | k_h1 | q_h2 | q_h3 | k_h2 | k_h3]
        cols = []
        bqk_rows = []
        for pair in range(2):
            h0, h1 = hs[2 * pair], hs[2 * pair + 1]
            cols += [wq[:, HD * h0:HD * (h0 + 1)] * sc,
                     wq[:, HD * h1:HD * (h1 + 1)] * sc]
            bqk_rows.append(np.concatenate(
                [bq[HD * h0:HD * (h0 + 1)], bq[HD * h1:HD * (h1 + 1)]]) * sc)
            cols += [wk[:, HD * h0:HD * (h0 + 1)],
                     wk[:, HD * h1:HD * (h1 + 1)]]
            bqk_rows.append(np.concatenate(
                [bk[HD * h0:HD * (h0 + 1)], bk[HD * h1:HD * (h1 + 1)]]))
        wqk_c = np.concatenate(cols, axis=1).astype(_BF16)
        bqk_c = np.stack(bqk_rows).astype(_BF16)

        wv_c = np.zeros((D, 260), dtype=np.float32)
        bv_c = np.zeros((1, 260), dtype=np.float32)
        for i, h in enumerate(hs):
            wv_c[:, 65 * i:65 * i + 64] = wv[:, HD * h:HD * (h + 1)]
            bv_c[0, 65 * i:65 * i + 64] = bv[HD * h:HD * (h + 1)]

        erb_c = np.empty((HPC, S, S), dtype=_BF16)
        for i, h in enumerate(hs):
            erb_c[i] = (np.exp(rel_bias[h].T) * tri).astype(_BF16)

        in_maps.append({
            "xT": xT[b],
            "wqk": wqk_c,
            "wv": wv_c.astype(_BF16),
            "bqk": bqk_c,
            "bv": bv_c.astype(_BF16),
            "erb": erb_c,
            "wo": np.ascontiguousarray(
                w_out[256 * hg:256 * (hg + 1)].reshape(2, 128, D)).astype(_BF16),
        })

    res = run_bass_kernel_spmd(nc, in_maps, list(range(NCORES)), trace=trace)
    _last_results = res

    out = np.zeros((B, S, D), dtype=np.float32)
    for c in range(NCORES):
        out[c // 4] += res.results[c]["out"]
    out += b_out
    return out



# revision 7
# speedup vs baseline: 1.1285x; 1.0226x over previous
"""Causal multi-head attention with relative position bias on 8 Trainium2
NeuronCores.

Problem (full shapes): x[2,2048,1024], rel_bias[16,2048,2048],
w_qkv[1024,3072], b_qkv[3072], w_out[1024,1024], b_out[1024].

Sharding: core = (batch, head-group): 2 batches x 4 head-groups of 4 heads.
Each core computes q/k/v projections for its 4 heads, causal attention with
rel-bias, and a partial output projection through its heads' rows of w_out.
Host sums the 4 partial outputs per batch (the tensor-parallel reduce) and
adds b_out.

Device kernel design notes (v2):
- Scores are computed TRANSPOSED (scoresT[kj,qi] = k.q): softmax reduction
  over keys is a matmul contraction (ones column in the PV stationary) and
  the PV matmul directly yields the transposed attention output the
  out-projection needs as stationary.
- A head PAIR shares each [128, 2, 512] score tile: one exp (ACT) and one
  multiply (DVE/GPSIMD) cover both heads, halving per-instruction overhead.
  exp(score + bias) = exp(score) * exp(bias): the host bakes exp(rel_biasT)
  with the causal mask as exact zeros, pair-packed to match.
- Causal clipping at 128 granularity: for key block kj only queries
  qi >= 128*kj are computed (partial-width matmuls/exp/mul), saving ~15%
  of attention-phase work versus 512-granular windows.
- PV stationary is [v_even | ones | v_odd] ([128,129]): one matmul per head
  produces 64 attention rows plus the softmax denominator row for free.
- Denominators: copied out of PSUM (IEEE layout needed), one
  reciprocal_approx_fast per chunk pair, broadcast across partitions via a
  DRAM stride-0 bounce.
- The PE stream is software-pipelined: score matmuls run 2 kj-blocks ahead
  of the PV matmuls, and lagged out-projection chains are interleaved into
  the attention stream so the tensor engine never starves (HAM stays warm)
  and the 8MB output DMA is spread across the attention phase.
"""

import math
import sys
import types
from collections import deque
from contextlib import ExitStack

import ml_dtypes
import numpy as np

B, S, D = 2, 2048, 1024
NH, HD = 16, 64
NCORES = 8
HPC = 4  # heads per core (2 pairs)

_BF16 = ml_dtypes.bfloat16

KC = D // 128  # 8 contraction chunks for the projections
NCH = S // 512  # 4 query chunks of 512 per head pair
NSC = S // 128  # 16 s-chunks


def _install_ntff_hook():
    """concourse.bass_utils imports antenv.axon_hooks for NTFF tracing under
    axon; this container's antenv lacks that module. Provide it, backed by
    the ctypes hook from trn_agent_boot (if present)."""
    if "antenv.axon_hooks" in sys.modules:
        return
    try:
        import antenv
    except ImportError:
        return
    mod = types.ModuleType("antenv.axon_hooks")
    mod._hook = None
    mod.set_axon_ntff_profile_hook = lambda h: setattr(mod, "_hook", h)
    mod.get_axon_ntff_profile_hook = lambda: mod._hook
    sys.modules["antenv.axon_hooks"] = mod
    antenv.axon_hooks = mod
    try:
        from trn_agent_boot.trn_boot import _ntff_profile_via_ctypes

        h = _ntff_profile_via_ctypes("/opt/axon/libaxon_pjrt.so")
        if h is not None:
            mod._hook = h
    except Exception:
        pass


def _phase_load(ctx, tc, nc, d, has_bqk, has_bv, st):
    """DMA weights + xT into persistent SBUF tiles; create v/qkT/attnT."""
    from concourse import mybir
    bf = mybir.dt.bfloat16

    xt_pool = ctx.enter_context(tc.tile_pool(name="xt", bufs=KC))
    wqk_pool = ctx.enter_context(tc.tile_pool(name="wqk", bufs=KC))
    wv_pool = ctx.enter_context(tc.tile_pool(name="wv", bufs=KC))
    wo_pool = ctx.enter_context(tc.tile_pool(name="wo", bufs=2))
    const_pool = ctx.enter_context(tc.tile_pool(name="consts", bufs=1))
    qkT_pool = ctx.enter_context(tc.tile_pool(name="qkT", bufs=4))
    v_pool = ctx.enter_context(tc.tile_pool(name="vsb", bufs=2 * NSC))
    attnT_pool = ctx.enter_context(tc.tile_pool(name="attnT", bufs=2))

    st.qkT_t = [qkT_pool.tile([128, S], bf, name="qkT", tag="qkT")
                for _ in range(4)]
    st.attnT_t = [attnT_pool.tile([128, S], bf, name="attnT", tag="attnT")
                  for _ in range(2)]
    # v_sb[pair][si]: [v_even(0:64) | 1 | v_odd(65:129) | 1] so both heads'
    # PV stationary slices ([0:65] and [65:130]) put attention at rows 0-63
    # and the softmax denominator at row 64 (engine APs need 32-aligned
    # partition starts, so the denominator cannot land on row 0 of the odd
    # head with a leading-ones layout)
    st.v_sb = [[v_pool.tile([128, 130], bf, name="vsb", tag="vsb")
                for _ in range(NSC)] for _ in range(2)]
    for pair in range(2):
        for si in range(NSC):
            nc.gpsimd.memset(st.v_sb[pair][si][:, 64:65], 1.0)
            nc.gpsimd.memset(st.v_sb[pair][si][:, 129:130], 1.0)

    if has_bqk or has_bv:
        st.ones_row = const_pool.tile([1, 512], bf)
        nc.gpsimd.memset(st.ones_row[:], 1.0)

    st.wqk_t, st.xt_t, st.wv_t = [], [], []
    for k in range(KC):
        w = wqk_pool.tile([128, 512], bf)
        nc.sync.dma_start(w[:], d.wqk[k * 128:(k + 1) * 128, :])
        st.wqk_t.append(w)
        xt = xt_pool.tile([128, S], bf)
        nc.sync.dma_start(xt[:], d.xT[k * 128:(k + 1) * 128, :])
        st.xt_t.append(xt)
    for k in range(KC):
        # wv is first consumed well into the projection phase; keep it out
        # of the critical DMA prefix the first qk chain waits on
        wv = wv_pool.tile([128, 256], bf)
        nc.sync.dma_start(wv[:], d.wv[k * 128:(k + 1) * 128, :])
        st.wv_t.append(wv)
    st.wo_t = []
    for p in range(2):
        w = wo_pool.tile([128, D], bf)
        nc.sync.dma_start(w[:], d.wo[p])
        st.wo_t.append(w)
    if has_bqk:
        st.bqk_sb = []
        for m in range(4):
            t = const_pool.tile([1, 128], bf, name=f"bqk{m}", tag=f"bqk{m}")
            nc.sync.dma_start(t[:], d.bqk[m:m + 1, :])
            st.bqk_sb.append(t)
    if has_bv:
        st.bv_sb = const_pool.tile([1, 256], bf)
        nc.sync.dma_start(st.bv_sb[:], d.bv[:])


def _phase_proj(ctx, tc, nc, has_bqk, has_bv, st):
    """qkv projections.

    qkT[m][r, s]: m-chunks 0..3 = [q pair0 | k pair0 | q pair1 | k pair1];
    within a chunk rows 0-63 = even head of the pair, 64-127 = odd head.
    v_sb[pair][si]: [128, 129] bf16 = [v_even | ones | v_odd].
    """
    from concourse import mybir
    f32 = mybir.dt.float32

    with tc.tile_pool(name="qk_ps", bufs=4, space="PSUM") as qk_ps, \
         tc.tile_pool(name="v_ps", bufs=3, space="PSUM") as v_ps:

        def emit_qk(m):
            for s4 in range(4):
                ps = qk_ps.tile([128, 512], f32, name="qkps", tag="qkps")
                for k in range(KC):
                    nc.tensor.matmul(
                        ps[:],
                        st.wqk_t[k][:, m * 128:(m + 1) * 128],
                        st.xt_t[k][:, s4 * 512:(s4 + 1) * 512],
                        start=(k == 0),
                        stop=(k == KC - 1 and not has_bqk),
                    )
                if has_bqk:
                    nc.tensor.matmul(
                        ps[:], st.bqk_sb[m][:], st.ones_row[:, :],
                        start=False, stop=True,
                    )
                nc.vector.tensor_copy(
                    st.qkT_t[m][:, s4 * 512:(s4 + 1) * 512], ps[:])

        def emit_v(pair):
            for si in range(NSC):
                ps = v_ps.tile([128, 128], f32, name="vps", tag="vps")
                for k in range(KC):
                    nc.tensor.matmul(
                        ps[:],
                        st.xt_t[k][:, si * 128:(si + 1) * 128],
                        st.wv_t[k][:, pair * 128:(pair + 1) * 128],
                        start=(k == 0),
                        stop=(k == KC - 1 and not has_bv),
                    )
                if has_bv:
                    nc.tensor.matmul(
                        ps[:], st.ones_row[0:1, 0:128],
                        st.bv_sb[0:1, pair * 128:(pair + 1) * 128],
                        start=False, stop=True,
                    )
                t = st.v_sb[pair][si]
                nc.scalar.copy(t[:, 0:64], ps[:, 0:64])
                nc.scalar.copy(t[:, 65:129], ps[:, 64:128])

        emit_qk(0)
        emit_qk(1)
        emit_v(0)
        emit_qk(2)
        emit_qk(3)
        emit_v(1)


def _phase_attn_out(ctx, tc, nc, d, st):
    from concourse import mybir
    bf = mybir.dt.bfloat16
    f32 = mybir.dt.float32
    EXP = mybir.ActivationFunctionType.Exp

    with ExitStack() as cctx:
        sc_ps = cctx.enter_context(tc.tile_pool(name="sc_ps", bufs=2, space="PSUM"))
        pv_ps = cctx.enter_context(tc.tile_pool(name="pv_ps", bufs=3, space="PSUM"))
        o_ps = cctx.enter_context(tc.tile_pool(name="o_ps", bufs=1, space="PSUM"))
        rb_pool = cctx.enter_context(tc.tile_pool(name="erb", bufs=8))
        esc_pool = cctx.enter_context(tc.tile_pool(name="esc", bufs=5))
        pr_pool = cctx.enter_context(tc.tile_pool(name="prob", bufs=5))
        rec_pool = cctx.enter_context(tc.tile_pool(name="rec", bufs=3))
        bc_pool = cctx.enter_context(tc.tile_pool(name="bc", bufs=3))
        dram_pool = cctx.enter_context(tc.tile_pool(name="recd", bufs=6, space="DRAM"))
        osb_pool = cctx.enter_context(tc.tile_pool(name="osb", bufs=4))

        out_q = deque()  # lagged out-projection chains (si, e2)

        def emit_out_chain():
            if not out_q:
                return
            si, e2 = out_q.popleft()
            ps = o_ps.tile([128, 512], f32, name="ops", tag="ops")
            for p in range(2):
                nc.tensor.matmul(
                    ps[:],
                    st.attnT_t[p][:, si * 128:(si + 1) * 128],
                    st.wo_t[p][:, e2 * 512:(e2 + 1) * 512],
                    start=(p == 0), stop=(p == 1),
                )
            osb = osb_pool.tile([128, 512], bf, name="osb", tag="osb")
            nc.vector.tensor_copy(osb[:], ps[:])
            nc.sync.dma_start(
                d.out[si * 128:(si + 1) * 128, e2 * 512:(e2 + 1) * 512],
                osb[:])

        def emit_attn_chunk(p, c):
            qT = st.qkT_t[2 * p]
            kT = st.qkT_t[2 * p + 1]
            nkj = 4 * (c + 1)
            pv_e = pv_ps.tile([65, 512], f32, name="pv", tag="pv")
            pv_o = pv_ps.tile([65, 512], f32, name="pv", tag="pv")
            pend = deque()

            def flush_pv():
                kjb, o, w, pr = pend.popleft()
                vt = st.v_sb[p][kjb]
                nc.tensor.matmul(
                    pv_e[0:65, o:o + w], vt[:, 0:65], pr[:, 0, o:o + w],
                    start=(kjb == 0), stop=(kjb == nkj - 1))
                nc.tensor.matmul(
                    pv_o[0:65, o:o + w], vt[:, 65:130], pr[:, 1, o:o + w],
                    start=(kjb == 0), stop=(kjb == nkj - 1))

            for kjb in range(nkj):
                o = max(0, kjb * 128 - c * 512)
                w = 512 - o
                q0 = c * 512 + o
                sc = sc_ps.tile([128, 2, 512], f32, name="sc", tag="sc")
                # both heads' score MMs adjacent: alternating PE row groups
                # let LDWEIGHTS overlap the in-flight matmul
                for h in range(2):
                    rows = slice(64 * h, 64 * h + 64)
                    nc.tensor.matmul(
                        sc[:, h, o:o + w],
                        kT[rows, kjb * 128:(kjb + 1) * 128],
                        qT[rows, q0:q0 + w],
                        start=True, stop=True,
                        tile_position=(64 * h, 0),
                    )
                esc = esc_pool.tile([128, 2, 512], bf, name="esc", tag="esc")
                nc.scalar.activation(esc[:, :, o:o + w], sc[:, :, o:o + w], EXP)
                rb = rb_pool.tile([128, 2, 512], bf, name="erb", tag="erb")
                nc.sync.dma_start(
                    rb[:, :, o:o + w],
                    d.erb[p, c, kjb * 128:(kjb + 1) * 128, :, o:o + w])
                pr = pr_pool.tile([128, 2, 512], bf, name="prob", tag="prob")
                # partial blocks go to the otherwise-idle GPSIMD, keeping the
                # DVE free for the full-width multiplies it does at 2x rate
                eng = nc.gpsimd if w <= 384 else nc.vector
                eng.tensor_mul(pr[:, :, o:o + w], esc[:, :, o:o + w],
                               rb[:, :, o:o + w])
                pend.append((kjb, o, w, pr))
                if len(pend) >= 3:
                    flush_pv()
                emit_out_chain()
            while pend:
                flush_pv()

            # normalization: both heads' denominators live in pv row 64; they
            # must leave PSUM before the bit-trick reciprocal (IEEE fp32) and
            # land on partition 0 (engine partition starts must be 32-aligned)
            den = rec_pool.tile([1, 1024], f32, name="den", tag="den")
            nc.vector.tensor_copy(den[0:1, 0:512], pv_e[64:65, :])
            nc.vector.tensor_copy(den[0:1, 512:1024], pv_o[64:65, :])
            rec = rec_pool.tile([1, 1024], f32, name="rec", tag="rec")
            nc.vector.reciprocal_approx_fast(out=rec[:], in_=den[:])
            dbc = dram_pool.tile([1, 1024], f32, name="recd", tag="recd")
            nc.sync.dma_start(dbc[:], rec[:])
            bc = bc_pool.tile([128, 512], f32, name="bc", tag="bc")
            nc.sync.dma_start(bc[0:64, :],
                              dbc[0:1, 0:512].partition_broadcast(64))
            nc.sync.dma_start(bc[64:128, :],
                              dbc[0:1, 512:1024].partition_broadcast(64))
            cs = c * 512
            nc.vector.tensor_mul(
                st.attnT_t[p][0:64, cs:cs + 512], pv_e[0:64, :], bc[0:64, :])
            nc.vector.tensor_mul(
                st.attnT_t[p][64:128, cs:cs + 512], pv_o[0:64, :], bc[64:128, :])

        for c in range(NCH):
            emit_attn_chunk(0, c)
            emit_attn_chunk(1, c)
            # out chains for chunk c-1 (both pairs done) interleave into the
            # next chunk's attention stream as PE filler
            if c > 0:
                for si in range(4 * (c - 1), 4 * c):
                    out_q.append((si, 0))
                    out_q.append((si, 1))
        for si in range(4 * (NCH - 1), 4 * NCH):
            out_q.append((si, 0))
            out_q.append((si, 1))
        while out_q:
            emit_out_chain()


_LDW_OPT_INSTALLED = False


def _enable_ldw_opt():
    """walrus ships with --enable-ldw-opt=false; flip it for this process
    (dedupes/hoists LDWEIGHTS). Gated by KERNEL_LDW_OPT=1."""
    global _LDW_OPT_INSTALLED
    if _LDW_OPT_INSTALLED:
        return
    _LDW_OPT_INSTALLED = True
    import os
    if os.environ.get("KERNEL_LDW_OPT", "0") != "1":
        return
    import concourse.bass_utils as bu
    orig = bu.run_command

    def patched(argv, **kwargs):
        argv = ["--enable-ldw-opt=true" if a == "--enable-ldw-opt=false" else a
                for a in argv]
        return orig(argv, **kwargs)

    bu.run_command = patched


def _build_program(has_bqk: bool, has_bv: bool):
    import concourse.tile as tile
    from concourse import bacc, mybir

    bf = mybir.dt.bfloat16
    f32 = mybir.dt.float32

    nc = bacc.Bacc("TRN2", target_bir_lowering=False, debug=False,
                   num_devices=NCORES)

    d = types.SimpleNamespace()
    d.xT = nc.dram_tensor("xT", [D, S], bf, kind="ExternalInput").ap()
    d.wqk = nc.dram_tensor("wqk", [D, 512], bf, kind="ExternalInput").ap()
    d.wv = nc.dram_tensor("wv", [D, 256], bf, kind="ExternalInput").ap()
    d.bqk = nc.dram_tensor("bqk", [4, 128], bf, kind="ExternalInput").ap()
    d.bv = nc.dram_tensor("bv", [1, 256], bf, kind="ExternalInput").ap()
    d.erb = nc.dram_tensor("erb", [2, NCH, S, 2, 512], bf,
                           kind="ExternalInput").ap()
    d.wo = nc.dram_tensor("wo", [2, 128, D], bf, kind="ExternalInput").ap()
    d.out = nc.dram_tensor("out", [S, D], bf, kind="ExternalOutput").ap()

    st = types.SimpleNamespace()
    with tile.TileContext(nc) as tc:
        with ExitStack() as ctx:
            _phase_load(ctx, tc, nc, d, has_bqk, has_bv, st)
            _phase_proj(ctx, tc, nc, has_bqk, has_bv, st)
            _phase_attn_out(ctx, tc, nc, d, st)

    nc.compile()
    return nc


_PROGRAM_CACHE = {}


def _get_program(has_bqk, has_bv):
    key = (has_bqk, has_bv)
    if key not in _PROGRAM_CACHE:
        _PROGRAM_CACHE[key] = _build_program(has_bqk, has_bv)
    return _PROGRAM_CACHE[key]


_last_results = None  # BassKernelResults of the most recent run (for test.py)


def kernel(x, rel_bias, w_qkv, b_qkv, w_out, b_out, *, trace=False):
    global _last_results
    _install_ntff_hook()
    _enable_ldw_opt()
    from concourse.bass_utils import run_bass_kernel_spmd

    x = np.asarray(x, dtype=np.float32)
    rel_bias = np.asarray(rel_bias, dtype=np.float32)
    w_qkv = np.asarray(w_qkv, dtype=np.float32)
    b_qkv = np.asarray(b_qkv, dtype=np.float32)
    w_out = np.asarray(w_out, dtype=np.float32)
    b_out = np.asarray(b_out, dtype=np.float32)

    wq = w_qkv[:, 0:D]
    wk = w_qkv[:, D:2 * D]
    wv = w_qkv[:, 2 * D:3 * D]
    bq, bk, bv = b_qkv[0:D], b_qkv[D:2 * D], b_qkv[2 * D:3 * D]
    has_bqk = bool(np.any(bq)) or bool(np.any(bk))
    has_bv = bool(np.any(bv))

    nc = _get_program(has_bqk, has_bv)

    sc = 1.0 / math.sqrt(HD)  # folded into the q projection
    xT = [np.ascontiguousarray(x[b].T).astype(_BF16) for b in range(B)]
    tri = np.triu(np.ones((S, S), dtype=np.float32))  # [kj, qi]: qi >= kj

    in_maps = []
    for c in range(NCORES):
        b, hg = divmod(c, 4)
        hs = [4 * hg + i for i in range(HPC)]

        # wqk columns: [q_h0 | q_h1 | k_h0 | k_h1 | q_h2 | q_h3 | k_h2 | k_h3]
        cols = []
        bqk_rows = []
        for pair in range(2):
            h0, h1 = hs[2 * pair], hs[2 * pair + 1]
            cols += [wq[:, HD * h0:HD * (h0 + 1)] * sc,
                     wq[:, HD * h1:HD * (h1 + 1)] * sc]
            bqk_rows.append(np.concatenate(
                [bq[HD * h0:HD * (h0 + 1)], bq[HD * h1:HD * (h1 + 1)]]) * sc)
            cols += [wk[:, HD * h0:HD * (h0 + 1)],
                     wk[:, HD * h1:HD * (h1 + 1)]]
            bqk_rows.append(np.concatenate(
                [bk[HD * h0:HD * (h0 + 1)], bk[HD * h1:HD * (h1 + 1)]]))
        wqk_c = np.concatenate(cols, axis=1).astype(_BF16)
        bqk_c = np.stack(bqk_rows).astype(_BF16)

        wv_c = np.zeros((D, 256), dtype=np.float32)
        bv_c = np.zeros((1, 256), dtype=np.float32)
        for pair in range(2):
            he, ho = hs[2 * pair], hs[2 * pair + 1]
            wv_c[:, pair * 128:pair * 128 + 64] = wv[:, HD * he:HD * (he + 1)]
            wv_c[:, pair * 128 + 64:pair * 128 + 128] = \
                wv[:, HD * ho:HD * (ho + 1)]
            bv_c[0, pair * 128:pair * 128 + 64] = bv[HD * he:HD * (he + 1)]
            bv_c[0, pair * 128 + 64:pair * 128 + 128] = \
                bv[HD * ho:HD * (ho + 1)]

        # erb[pair, chunk, kj, h, qi_in_chunk] = exp(rel_bias^T) * causal
        erb_c = np.empty((2, NCH, S, 2, 512), dtype=_BF16)
        for pair in range(2):
            for i_h in range(2):
                head = hs[2 * pair + i_h]
                m = (np.exp(rel_bias[head].T) * tri).astype(_BF16)
                erb_c[pair, :, :, i_h, :] = \
                    m.reshape(S, NCH, 512).transpose(1, 0, 2)

        in_maps.append({
            "xT": xT[b],
            "wqk": wqk_c,
            "wv": wv_c.astype(_BF16),
            "bqk": bqk_c,
            "bv": bv_c.astype(_BF16),
            "erb": erb_c,
            "wo": np.ascontiguousarray(
                w_out[256 * hg:256 * (hg + 1)].reshape(2, 128, D)).astype(_BF16),
        })

    res = run_bass_kernel_spmd(nc, in_maps, list(range(NCORES)), trace=trace)
    _last_results = res

    out = np.zeros((B, S, D), dtype=np.float32)
    for c in range(NCORES):
        out[c // 4] += np.asarray(res.results[c]["out"], dtype=np.float32)
    out += b_out
    return out


# revision 9
# speedup vs baseline: 1.1713x; 1.0380x over previous
"""Causal multi-head attention with relative position bias on 8 Trainium2
NeuronCores.

Problem (full shapes): x[2,2048,1024], rel_bias[16,2048,2048],
w_qkv[1024,3072], b_qkv[3072], w_out[1024,1024], b_out[1024].

Sharding: core = (batch, head-group): 2 batches x 4 head-groups of 4 heads.
Each core computes q/k/v projections for its 4 heads, causal attention with
rel-bias, and a partial output projection through its heads' rows of w_out.
Host sums the 4 partial outputs per batch (the tensor-parallel reduce) and
adds b_out.

Device kernel design notes (v2):
- Scores are computed TRANSPOSED (scoresT[kj,qi] = k.q): softmax reduction
  over keys is a matmul contraction (ones column in the PV stationary) and
  the PV matmul directly yields the transposed attention output the
  out-projection needs as stationary.
- A head PAIR shares each [128, 2, 512] score tile: one exp (ACT) and one
  multiply (DVE/GPSIMD) cover both heads, halving per-instruction overhead.
  exp(score + bias) = exp(score) * exp(bias): the host bakes exp(rel_biasT)
  with the causal mask as exact zeros, pair-packed to match.
- Causal clipping at 128 granularity: for key block kj only queries
  qi >= 128*kj are computed (partial-width matmuls/exp/mul), saving ~15%
  of attention-phase work versus 512-granular windows.
- PV stationary is [v_even | ones | v_odd] ([128,129]): one matmul per head
  produces 64 attention rows plus the softmax denominator row for free.
- Denominators: copied out of PSUM (IEEE layout needed), one
  reciprocal_approx_fast per chunk pair, broadcast across partitions via a
  DRAM stride-0 bounce.
- The PE stream is software-pipelined: score matmuls run 2 kj-blocks ahead
  of the PV matmuls, and lagged out-projection chains are interleaved into
  the attention stream so the tensor engine never starves (HAM stays warm)
  and the 8MB output DMA is spread across the attention phase.
"""

import math
import sys
import types
from collections import deque
from contextlib import ExitStack

import ml_dtypes
import numpy as np

B, S, D = 2, 2048, 1024
NH, HD = 16, 64
NCORES = 8
HPC = 4  # heads per core (2 pairs)

_BF16 = ml_dtypes.bfloat16

KC = D // 128  # 8 contraction chunks for the projections
NCH = S // 512  # 4 query chunks of 512 per head pair
NSC = S // 128  # 16 s-chunks


def _install_ntff_hook():
    """concourse.bass_utils imports antenv.axon_hooks for NTFF tracing under
    axon; this container's antenv lacks that module. Provide it, backed by
    the ctypes hook from trn_agent_boot (if present)."""
    if "antenv.axon_hooks" in sys.modules:
        return
    try:
        import antenv
    except ImportError:
        return
    mod = types.ModuleType("antenv.axon_hooks")
    mod._hook = None
    mod.set_axon_ntff_profile_hook = lambda h: setattr(mod, "_hook", h)
    mod.get_axon_ntff_profile_hook = lambda: mod._hook
    sys.modules["antenv.axon_hooks"] = mod
    antenv.axon_hooks = mod
    try:
        from trn_agent_boot.trn_boot import _ntff_profile_via_ctypes

        h = _ntff_profile_via_ctypes("/opt/axon/libaxon_pjrt.so")
        if h is not None:
            mod._hook = h
    except Exception:
        pass


def _phase_load(ctx, tc, nc, d, has_bqk, has_bv, st):
    """DMA weights + xT into persistent SBUF tiles; create v/qkT/attnT."""
    from concourse import mybir
    bf = mybir.dt.bfloat16

    xt_pool = ctx.enter_context(tc.tile_pool(name="xt", bufs=KC))
    wqk_pool = ctx.enter_context(tc.tile_pool(name="wqk", bufs=KC))
    wv_pool = ctx.enter_context(tc.tile_pool(name="wv", bufs=KC))
    wo_pool = ctx.enter_context(tc.tile_pool(name="wo", bufs=2))
    const_pool = ctx.enter_context(tc.tile_pool(name="consts", bufs=1))
    qkT_pool = ctx.enter_context(tc.tile_pool(name="qkT", bufs=4))
    v_pool = ctx.enter_context(tc.tile_pool(name="vsb", bufs=2 * NSC))
    attnT_pool = ctx.enter_context(tc.tile_pool(name="attnT", bufs=2))

    st.qkT_t = [qkT_pool.tile([128, S], bf, name="qkT", tag="qkT")
                for _ in range(4)]
    st.attnT_t = [attnT_pool.tile([128, S], bf, name="attnT", tag="attnT")
                  for _ in range(2)]
    # v_sb[pair][si]: [v_even(0:64) | 1 | v_odd(65:129) | 1] so both heads'
    # PV stationary slices ([0:65] and [65:130]) put attention at rows 0-63
    # and the softmax denominator at row 64 (engine APs need 32-aligned
    # partition starts, so the denominator cannot land on row 0 of the odd
    # head with a leading-ones layout)
    st.v_sb = [[v_pool.tile([128, 130], bf, name="vsb", tag="vsb")
                for _ in range(NSC)] for _ in range(2)]
    for pair in range(2):
        for si in range(NSC):
            nc.gpsimd.memset(st.v_sb[pair][si][:, 64:65], 1.0)
            nc.gpsimd.memset(st.v_sb[pair][si][:, 129:130], 1.0)

    if has_bqk or has_bv:
        st.ones_row = const_pool.tile([1, 512], bf)
        nc.gpsimd.memset(st.ones_row[:], 1.0)

    st.wqk_t, st.xt_t, st.wv_t = [], [], []
    for k in range(KC):
        w = wqk_pool.tile([128, 512], bf)
        nc.sync.dma_start(w[:], d.wqk[k * 128:(k + 1) * 128, :])
        st.wqk_t.append(w)
        xt = xt_pool.tile([128, S], bf)
        nc.sync.dma_start(xt[:], d.xT[k * 128:(k + 1) * 128, :])
        st.xt_t.append(xt)
    for k in range(KC):
        # wv is first consumed well into the projection phase; keep it out
        # of the critical DMA prefix the first qk chain waits on
        wv = wv_pool.tile([128, 256], bf)
        nc.sync.dma_start(wv[:], d.wv[k * 128:(k + 1) * 128, :])
        st.wv_t.append(wv)
    st.wo_t = []
    for p in range(2):
        w = wo_pool.tile([128, D], bf)
        nc.sync.dma_start(w[:], d.wo[p])
        st.wo_t.append(w)
    if has_bqk:
        st.bqk_sb = []
        for m in range(4):
            t = const_pool.tile([1, 128], bf, name=f"bqk{m}", tag=f"bqk{m}")
            nc.sync.dma_start(t[:], d.bqk[m:m + 1, :])
            st.bqk_sb.append(t)
    if has_bv:
        st.bv_sb = const_pool.tile([1, 256], bf)
        nc.sync.dma_start(st.bv_sb[:], d.bv[:])


def _phase_proj(ctx, tc, nc, has_bqk, has_bv, st):
    """qkv projections.

    qkT[m][r, s]: m-chunks 0..3 = [q pair0 | k pair0 | q pair1 | k pair1];
    within a chunk rows 0-63 = even head of the pair, 64-127 = odd head.
    v_sb[pair][si]: [128, 129] bf16 = [v_even | ones | v_odd].
    """
    from concourse import mybir
    f32 = mybir.dt.float32

    with tc.tile_pool(name="qk_ps", bufs=4, space="PSUM") as qk_ps, \
         tc.tile_pool(name="v_ps", bufs=3, space="PSUM") as v_ps:

        def emit_qk(m):
            for s4 in range(4):
                ps = qk_ps.tile([128, 512], f32, name="qkps", tag="qkps")
                for k in range(KC):
                    nc.tensor.matmul(
                        ps[:],
                        st.wqk_t[k][:, m * 128:(m + 1) * 128],
                        st.xt_t[k][:, s4 * 512:(s4 + 1) * 512],
                        start=(k == 0),
                        stop=(k == KC - 1 and not has_bqk),
                    )
                if has_bqk:
                    nc.tensor.matmul(
                        ps[:], st.bqk_sb[m][:], st.ones_row[:, :],
                        start=False, stop=True,
                    )
                nc.vector.tensor_copy(
                    st.qkT_t[m][:, s4 * 512:(s4 + 1) * 512], ps[:])

        def emit_v(pair):
            for si in range(NSC):
                ps = v_ps.tile([128, 128], f32, name="vps", tag="vps")
                for k in range(KC):
                    nc.tensor.matmul(
                        ps[:],
                        st.xt_t[k][:, si * 128:(si + 1) * 128],
                        st.wv_t[k][:, pair * 128:(pair + 1) * 128],
                        start=(k == 0),
                        stop=(k == KC - 1 and not has_bv),
                    )
                if has_bv:
                    nc.tensor.matmul(
                        ps[:], st.ones_row[0:1, 0:128],
                        st.bv_sb[0:1, pair * 128:(pair + 1) * 128],
                        start=False, stop=True,
                    )
                t = st.v_sb[pair][si]
                nc.scalar.copy(t[:, 0:64], ps[:, 0:64])
                nc.scalar.copy(t[:, 65:129], ps[:, 64:128])

        emit_qk(0)
        emit_qk(1)
        emit_v(0)
        emit_qk(2)
        emit_qk(3)
        emit_v(1)


def _phase_attn_out(ctx, tc, nc, d, st):
    from concourse import mybir
    bf = mybir.dt.bfloat16
    f32 = mybir.dt.float32
    EXP = mybir.ActivationFunctionType.Exp

    with ExitStack() as cctx:
        # PSUM budget (8 banks): sc ring 3 x 2 banks + pv 2 x 1 bank. The
        # out-projection borrows sc-ring slots instead of its own pool; the
        # deep sc ring is what gives the PE enough runway to keep the HAM
        # clock-gate at 2.4GHz.
        sc_ps = cctx.enter_context(tc.tile_pool(name="sc_ps", bufs=3, space="PSUM"))
        pv_ps = cctx.enter_context(tc.tile_pool(name="pv_ps", bufs=2, space="PSUM"))
        rb_pool = cctx.enter_context(tc.tile_pool(name="erb", bufs=8))
        esc_pool = cctx.enter_context(tc.tile_pool(name="esc", bufs=5))
        pr_pool = cctx.enter_context(tc.tile_pool(name="prob", bufs=5))
        pvf_pool = cctx.enter_context(tc.tile_pool(name="pvf", bufs=4))
        pk_pool = cctx.enter_context(tc.tile_pool(name="pk", bufs=3))
        bc_pool = cctx.enter_context(tc.tile_pool(name="bc", bufs=3))
        dram_pool = cctx.enter_context(tc.tile_pool(name="recd", bufs=6, space="DRAM"))
        osb_pool = cctx.enter_context(tc.tile_pool(name="osb", bufs=4))

        out_q = deque()  # lagged out-projection chains (si, e2)
        mul_rr = [0]  # round-robin counter for DVE/GPSIMD mul split

        def emit_out_chain():
            if not out_q:
                return
            si, e2 = out_q.popleft()
            ps = sc_ps.tile([128, 2, 512], f32, name="sc", tag="sc")
            for p in range(2):
                nc.tensor.matmul(
                    ps[:, 0, :],
                    st.attnT_t[p][:, si * 128:(si + 1) * 128],
                    st.wo_t[p][:, e2 * 512:(e2 + 1) * 512],
                    start=(p == 0), stop=(p == 1),
                )
            osb = osb_pool.tile([128, 512], bf, name="osb", tag="osb")
            if (si + e2) % 2:
                nc.scalar.copy(osb[:], ps[:, 0, :])
            else:
                nc.vector.tensor_copy(osb[:], ps[:, 0, :])
            nc.sync.dma_start(
                d.out[si * 128:(si + 1) * 128, e2 * 512:(e2 + 1) * 512],
                osb[:])

        def emit_attn_chunk(p, c):
            qT = st.qkT_t[2 * p]
            kT = st.qkT_t[2 * p + 1]
            nkj = 4 * (c + 1)
            pv_e = pv_ps.tile([65, 512], f32, name="pv", tag="pv")
            pv_o = pv_ps.tile([65, 512], f32, name="pv", tag="pv")
            pend = deque()

            def flush_pv():
                kjb, o, w, pr = pend.popleft()
                vt = st.v_sb[p][kjb]
                nc.tensor.matmul(
                    pv_e[0:65, o:o + w], vt[:, 0:65], pr[:, 0, o:o + w],
                    start=(kjb == 0), stop=(kjb == nkj - 1))
                nc.tensor.matmul(
                    pv_o[0:65, o:o + w], vt[:, 65:130], pr[:, 1, o:o + w],
                    start=(kjb == 0), stop=(kjb == nkj - 1))

            for kjb in range(nkj):
                o = max(0, kjb * 128 - c * 512)
                w = 512 - o
                q0 = c * 512 + o
                sc = sc_ps.tile([128, 2, 512], f32, name="sc", tag="sc")
                # both heads' score MMs adjacent: alternating PE row groups
                # let LDWEIGHTS overlap the in-flight matmul
                for h in range(2):
                    rows = slice(64 * h, 64 * h + 64)
                    nc.tensor.matmul(
                        sc[:, h, o:o + w],
                        kT[rows, kjb * 128:(kjb + 1) * 128],
                        qT[rows, q0:q0 + w],
                        start=True, stop=True,
                        tile_position=(64 * h, 0),
                    )
                esc = esc_pool.tile([128, 2, 512], bf, name="esc", tag="esc")
                nc.scalar.activation(esc[:, :, o:o + w], sc[:, :, o:o + w], EXP)
                rb = rb_pool.tile([128, 2, 512], bf, name="erb", tag="erb")
                nc.sync.dma_start(
                    rb[:, :, o:o + w],
                    d.erb[p, c, kjb * 128:(kjb + 1) * 128, :, o:o + w])
                pr = pr_pool.tile([128, 2, 512], bf, name="prob", tag="prob")
                # partial blocks and every 4th full block go to the
                # otherwise-idle GPSIMD to keep the DVE off the critical path
                if w <= 384:
                    eng = nc.gpsimd
                else:
                    mul_rr[0] += 1
                    eng = nc.gpsimd if mul_rr[0] % 4 == 0 else nc.vector
                eng.tensor_mul(pr[:, :, o:o + w], esc[:, :, o:o + w],
                               rb[:, :, o:o + w])
                pend.append((kjb, o, w, pr))
                if len(pend) >= 3:
                    flush_pv()
                emit_out_chain()
            while pend:
                flush_pv()

            # evict both pv accumulators to SBUF immediately (bf16; the
            # rounding is relative so it cancels in the normalization) so the
            # 2-deep pv ring never stalls the next chunk's matmuls
            pvf_e = pvf_pool.tile([65, 512], bf, name="pvf", tag="pvf")
            pvf_o = pvf_pool.tile([65, 512], bf, name="pvf", tag="pvf")
            nc.scalar.copy(pvf_e[:], pv_e[:])
            nc.vector.tensor_copy(pvf_o[:], pv_o[:])

            # denominators (pvf row 64): pack 2x[1,512] into [64,16] via an
            # SBUF->SBUF DMA so the cast+reciprocal run 64-partition-parallel
            pk_b = pk_pool.tile([64, 16], bf, name="pkb", tag="pkb")
            nc.sync.dma_start(pk_b[0:32, :], pvf_e[64:65, :])
            nc.sync.dma_start(pk_b[32:64, :], pvf_o[64:65, :])
            pk_f = pk_pool.tile([64, 16], f32, name="pkf", tag="pkf")
            nc.vector.tensor_copy(pk_f[:], pk_b[:])
            rec = pk_pool.tile([64, 16], f32, name="rec", tag="rec")
            nc.vector.reciprocal_approx_fast(out=rec[:], in_=pk_f[:])
            dbc = dram_pool.tile([2, 512], f32, name="recd", tag="recd")
            nc.sync.dma_start(dbc[:], rec[:])
            # both halves at base partition 0: SBUF/SBUF tensor_tensor inputs
            # must share their base partition
            bc = bc_pool.tile([64, 1024], f32, name="bc", tag="bc")
            nc.sync.dma_start(bc[:, 0:512],
                              dbc[0:1, :].partition_broadcast(64))
            nc.sync.dma_start(bc[:, 512:1024],
                              dbc[1:2, :].partition_broadcast(64))
            cs = c * 512
            nc.vector.tensor_mul(
                st.attnT_t[p][0:64, cs:cs + 512], pvf_e[0:64, :],
                bc[:, 0:512])
            nc.vector.tensor_mul(
                st.attnT_t[p][64:128, cs:cs + 512], pvf_o[0:64, :],
                bc[:, 512:1024])

        # big chunks first: out chains for group c interleave into the next
        # (smaller) group as PE filler, and the final group is the smallest
        order = list(range(NCH - 1, -1, -1))
        for gi, c in enumerate(order):
            emit_attn_chunk(0, c)
            emit_attn_chunk(1, c)
            if gi > 0:
                cprev = order[gi - 1]
                for si in range(4 * cprev, 4 * cprev + 4):
                    out_q.append((si, 0))
                    out_q.append((si, 1))
        for si in range(0, 4):
            out_q.append((si, 0))
            out_q.append((si, 1))
        while out_q:
            emit_out_chain()


_LDW_OPT_INSTALLED = False


def _enable_ldw_opt():
    """walrus ships with --enable-ldw-opt=false; flip it for this process
    (dedupes/hoists LDWEIGHTS). Gated by KERNEL_LDW_OPT=1."""
    global _LDW_OPT_INSTALLED
    if _LDW_OPT_INSTALLED:
        return
    _LDW_OPT_INSTALLED = True
    import os
    if os.environ.get("KERNEL_LDW_OPT", "0") != "1":
        return
    import concourse.bass_utils as bu
    orig = bu.run_command

    def patched(argv, **kwargs):
        argv = ["--enable-ldw-opt=true" if a == "--enable-ldw-opt=false" else a
                for a in argv]
        return orig(argv, **kwargs)

    bu.run_command = patched


def _build_program(has_bqk: bool, has_bv: bool):
    import concourse.tile as tile
    from concourse import bacc, mybir

    bf = mybir.dt.bfloat16
    f32 = mybir.dt.float32

    nc = bacc.Bacc("TRN2", target_bir_lowering=False, debug=False,
                   num_devices=NCORES)

    d = types.SimpleNamespace()
    d.xT = nc.dram_tensor("xT", [D, S], bf, kind="ExternalInput").ap()
    d.wqk = nc.dram_tensor("wqk", [D, 512], bf, kind="ExternalInput").ap()
    d.wv = nc.dram_tensor("wv", [D, 256], bf, kind="ExternalInput").ap()
    d.bqk = nc.dram_tensor("bqk", [4, 128], bf, kind="ExternalInput").ap()
    d.bv = nc.dram_tensor("bv", [1, 256], bf, kind="ExternalInput").ap()
    d.erb = nc.dram_tensor("erb", [2, NCH, S, 2, 512], bf,
                           kind="ExternalInput").ap()
    d.wo = nc.dram_tensor("wo", [2, 128, D], bf, kind="ExternalInput").ap()
    d.out = nc.dram_tensor("out", [S, D], bf, kind="ExternalOutput").ap()

    st = types.SimpleNamespace()
    with tile.TileContext(nc) as tc:
        with ExitStack() as ctx:
            _phase_load(ctx, tc, nc, d, has_bqk, has_bv, st)
            _phase_proj(ctx, tc, nc, has_bqk, has_bv, st)
            _phase_attn_out(ctx, tc, nc, d, st)

    nc.compile()
    return nc


_PROGRAM_CACHE = {}


def _get_program(has_bqk, has_bv):
    key = (has_bqk, has_bv)
    if key not in _PROGRAM_CACHE:
        _PROGRAM_CACHE[key] = _build_program(has_bqk, has_bv)
    return _PROGRAM_CACHE[key]


_last_results = None  # BassKernelResults of the most recent run (for test.py)


def kernel(x, rel_bias, w_qkv, b_qkv, w_out, b_out, *, trace=False):
    global _last_results
    _install_ntff_hook()
    _enable_ldw_opt()
    from concourse.bass_utils import run_bass_kernel_spmd

    x = np.asarray(x, dtype=np.float32)
    rel_bias = np.asarray(rel_bias, dtype=np.float32)
    w_qkv = np.asarray(w_qkv, dtype=np.float32)
    b_qkv = np.asarray(b_qkv, dtype=np.float32)
    w_out = np.asarray(w_out, dtype=np.float32)
    b_out = np.asarray(b_out, dtype=np.float32)

    wq = w_qkv[:, 0:D]
    wk = w_qkv[:, D:2 * D]
    wv = w_qkv[:, 2 * D:3 * D]
    bq, bk, bv = b_qkv[0:D], b_qkv[D:2 * D], b_qkv[2 * D:3 * D]
    has_bqk = bool(np.any(bq)) or bool(np.any(bk))
    has_bv = bool(np.any(bv))

    nc = _get_program(has_bqk, has_bv)

    sc = 1.0 / math.sqrt(HD)  # folded into the q projection
    xT = [np.ascontiguousarray(x[b].T).astype(_BF16) for b in range(B)]
    tri = np.triu(np.ones((S, S), dtype=np.float32))  # [kj, qi]: qi >= kj

    in_maps = []
    for c in range(NCORES):
        b, hg = divmod(c, 4)
        hs = [4 * hg + i for i in range(HPC)]

        # wqk columns: [q_h0 | q_h1 | k_h0 | k_h1 | q_h2 | q_h3 | k_h2 | k_h3]
        cols = []
        bqk_rows = []
        for pair in range(2):
            h0, h1 = hs[2 * pair], hs[2 * pair + 1]
            cols += [wq[:, HD * h0:HD * (h0 + 1)] * sc,
                     wq[:, HD * h1:HD * (h1 + 1)] * sc]
            bqk_rows.append(np.concatenate(
                [bq[HD * h0:HD * (h0 + 1)], bq[HD * h1:HD * (h1 + 1)]]) * sc)
            cols += [wk[:, HD * h0:HD * (h0 + 1)],
                     wk[:, HD * h1:HD * (h1 + 1)]]
            bqk_rows.append(np.concatenate(
                [bk[HD * h0:HD * (h0 + 1)], bk[HD * h1:HD * (h1 + 1)]]))
        wqk_c = np.concatenate(cols, axis=1).astype(_BF16)
        bqk_c = np.stack(bqk_rows).astype(_BF16)

        wv_c = np.zeros((D, 256), dtype=np.float32)
        bv_c = np.zeros((1, 256), dtype=np.float32)
        for pair in range(2):
            he, ho = hs[2 * pair], hs[2 * pair + 1]
            wv_c[:, pair * 128:pair * 128 + 64] = wv[:, HD * he:HD * (he + 1)]
            wv_c[:, pair * 128 + 64:pair * 128 + 128] = \
                wv[:, HD * ho:HD * (ho + 1)]
            bv_c[0, pair * 128:pair * 128 + 64] = bv[HD * he:HD * (he + 1)]
            bv_c[0, pair * 128 + 64:pair * 128 + 128] = \
                bv[HD * ho:HD * (ho + 1)]

        # erb[pair, chunk, kj, h, qi_in_chunk] = exp(rel_bias^T) * causal
        erb_c = np.empty((2, NCH, S, 2, 512), dtype=_BF16)
        for pair in range(2):
            for i_h in range(2):
                head = hs[2 * pair + i_h]
                m = (np.exp(rel_bias[head].T) * tri).astype(_BF16)
                erb_c[pair, :, :, i_h, :] = \
                    m.reshape(S, NCH, 512).transpose(1, 0, 2)

        in_maps.append({
            "xT": xT[b],
            "wqk": wqk_c,
            "wv": wv_c.astype(_BF16),
            "bqk": bqk_c,
            "bv": bv_c.astype(_BF16),
            "erb": erb_c,
            "wo": np.ascontiguousarray(
                w_out[256 * hg:256 * (hg + 1)].reshape(2, 128, D)).astype(_BF16),
        })

    res = run_bass_kernel_spmd(nc, in_maps, list(range(NCORES)), trace=trace)
    _last_results = res

    out = np.zeros((B, S, D), dtype=np.float32)
    for c in range(NCORES):
        out[c // 4] += np.asarray(res.results[c]["out"], dtype=np.float32)
    out += b_out
    return out


# revision 16
# speedup vs baseline: 1.2659x; 1.0808x over previous
"""Causal multi-head attention with relative position bias on 8 Trainium2
NeuronCores.

Problem (full shapes): x[2,2048,1024], rel_bias[16,2048,2048],
w_qkv[1024,3072], b_qkv[3072], w_out[1024,1024], b_out[1024].

Sharding: core = (batch, head-group): 2 batches x 4 head-groups of 4 heads.
Each core computes q/k/v projections for its 4 heads, causal attention with
rel-bias, and a partial output projection through its heads' rows of w_out.
Host sums the 4 partial outputs per batch (the tensor-parallel reduce) and
adds b_out.

Device kernel design notes (v2):
- Scores are computed TRANSPOSED (scoresT[kj,qi] = k.q): softmax reduction
  over keys is a matmul contraction (ones column in the PV stationary) and
  the PV matmul directly yields the transposed attention output the
  out-projection needs as stationary.
- A head PAIR shares each [128, 2, 512] score tile: one exp (ACT) and one
  multiply (DVE/GPSIMD) cover both heads, halving per-instruction overhead.
  exp(score + bias) = exp(score) * exp(bias): the host bakes exp(rel_biasT)
  with the causal mask as exact zeros, pair-packed to match.
- Causal clipping at 128 granularity: for key block kj only queries
  qi >= 128*kj are computed (partial-width matmuls/exp/mul), saving ~15%
  of attention-phase work versus 512-granular windows.
- PV stationary is [v_even | ones | v_odd] ([128,129]): one matmul per head
  produces 64 attention rows plus the softmax denominator row for free.
- Denominators: copied out of PSUM (IEEE layout needed), one
  reciprocal_approx_fast per chunk pair, broadcast across partitions via a
  DRAM stride-0 bounce.
- The PE stream is software-pipelined: score matmuls run 2 kj-blocks ahead
  of the PV matmuls, and lagged out-projection chains are interleaved into
  the attention stream so the tensor engine never starves (HAM stays warm)
  and the 8MB output DMA is spread across the attention phase.
"""

import math
import sys
import types
from collections import deque
from contextlib import ExitStack

import ml_dtypes
import numpy as np

B, S, D = 2, 2048, 1024
NH, HD = 16, 64
NCORES = 8
HPC = 4  # heads per core (2 pairs)

_BF16 = ml_dtypes.bfloat16

KC = D // 128  # 8 contraction chunks for the projections
NCH = S // 512  # 4 query chunks of 512 per head pair
NSC = S // 128  # 16 s-chunks


def _install_ntff_hook():
    """concourse.bass_utils imports antenv.axon_hooks for NTFF tracing under
    axon; this container's antenv lacks that module. Provide it, backed by
    the ctypes hook from trn_agent_boot (if present)."""
    if "antenv.axon_hooks" in sys.modules:
        return
    try:
        import antenv
    except ImportError:
        return
    mod = types.ModuleType("antenv.axon_hooks")
    mod._hook = None
    mod.set_axon_ntff_profile_hook = lambda h: setattr(mod, "_hook", h)
    mod.get_axon_ntff_profile_hook = lambda: mod._hook
    sys.modules["antenv.axon_hooks"] = mod
    antenv.axon_hooks = mod
    try:
        from trn_agent_boot.trn_boot import _ntff_profile_via_ctypes

        h = _ntff_profile_via_ctypes("/opt/axon/libaxon_pjrt.so")
        if h is not None:
            mod._hook = h
    except Exception:
        pass


def _phase_load(ctx, tc, nc, d, has_bqk, has_bv, st):
    """DMA weights + xT into persistent SBUF tiles; create v/qkT/attnT."""
    from concourse import mybir
    bf = mybir.dt.bfloat16

    xt_pool = ctx.enter_context(tc.tile_pool(name="xt", bufs=KC))
    wqk_pool = ctx.enter_context(tc.tile_pool(name="wqk", bufs=KC))
    wv_pool = ctx.enter_context(tc.tile_pool(name="wv", bufs=KC))
    wo_pool = ctx.enter_context(tc.tile_pool(name="wo", bufs=2))
    const_pool = ctx.enter_context(tc.tile_pool(name="consts", bufs=1))
    qkT_pool = ctx.enter_context(tc.tile_pool(name="qkT", bufs=4))
    v_pool = ctx.enter_context(tc.tile_pool(name="vsb", bufs=2 * NSC))
    attnT_pool = ctx.enter_context(tc.tile_pool(name="attnT", bufs=2))

    st.qkT_t = [qkT_pool.tile([128, S], bf, name="qkT", tag="qkT")
                for _ in range(4)]
    st.attnT_t = [attnT_pool.tile([128, S], bf, name="attnT", tag="attnT")
                  for _ in range(2)]
    # v_sb[pair][si]: [v_even(0:64) | 1 | v_odd(65:129) | 1] so both heads'
    # PV stationary slices ([0:65] and [65:130]) put attention at rows 0-63
    # and the softmax denominator at row 64 (engine APs need 32-aligned
    # partition starts, so the denominator cannot land on row 0 of the odd
    # head with a leading-ones layout)
    st.v_sb = [[v_pool.tile([128, 130], bf, name="vsb", tag="vsb")
                for _ in range(NSC)] for _ in range(2)]
    for pair in range(2):
        for si in range(NSC):
            nc.gpsimd.memset(st.v_sb[pair][si][:, 64:65], 1.0)
            nc.gpsimd.memset(st.v_sb[pair][si][:, 129:130], 1.0)

    if has_bqk or has_bv:
        st.ones_row = const_pool.tile([1, 512], bf)
        nc.gpsimd.memset(st.ones_row[:], 1.0)

    st.wqk_t, st.xt_t, st.wv_t = [], [], []
    for k in range(KC):
        w = wqk_pool.tile([128, 512], bf)
        nc.sync.dma_start(w[:], d.wqk[k * 128:(k + 1) * 128, :])
        st.wqk_t.append(w)
        xt = xt_pool.tile([128, S], bf)
        nc.sync.dma_start(xt[:], d.xT[k * 128:(k + 1) * 128, :])
        st.xt_t.append(xt)
    for k in range(KC):
        # wv is first consumed well into the projection phase; keep it out
        # of the critical DMA prefix the first qk chain waits on
        wv = wv_pool.tile([128, 256], bf)
        nc.sync.dma_start(wv[:], d.wv[k * 128:(k + 1) * 128, :])
        st.wv_t.append(wv)
    st.wo_t = []
    for p in range(2):
        w = wo_pool.tile([128, D], bf)
        nc.sync.dma_start(w[:], d.wo[p])
        st.wo_t.append(w)
    if has_bqk:
        st.bqk_sb = []
        for m in range(4):
            t = const_pool.tile([1, 128], bf, name=f"bqk{m}", tag=f"bqk{m}")
            nc.sync.dma_start(t[:], d.bqk[m:m + 1, :])
            st.bqk_sb.append(t)
    if has_bv:
        st.bv_sb = const_pool.tile([1, 256], bf)
        nc.sync.dma_start(st.bv_sb[:], d.bv[:])


def _phase_proj(ctx, tc, nc, has_bqk, has_bv, st):
    """qkv projections.

    qkT[m][r, s]: m-chunks 0..3 = [q pair0 | k pair0 | q pair1 | k pair1];
    within a chunk rows 0-63 = even head of the pair, 64-127 = odd head.
    v_sb[pair][si]: [128, 129] bf16 = [v_even | ones | v_odd].
    """
    from concourse import mybir
    f32 = mybir.dt.float32

    with tc.tile_pool(name="qk_ps", bufs=4, space="PSUM") as qk_ps, \
         tc.tile_pool(name="v_ps", bufs=3, space="PSUM") as v_ps:

        def emit_qk(m):
            for s4 in range(4):
                ps = qk_ps.tile([128, 512], f32, name="qkps", tag="qkps")
                for k in range(KC):
                    nc.tensor.matmul(
                        ps[:],
                        st.wqk_t[k][:, m * 128:(m + 1) * 128],
                        st.xt_t[k][:, s4 * 512:(s4 + 1) * 512],
                        start=(k == 0),
                        stop=(k == KC - 1 and not has_bqk),
                    )
                if has_bqk:
                    nc.tensor.matmul(
                        ps[:], st.bqk_sb[m][:], st.ones_row[:, :],
                        start=False, stop=True,
                    )
                nc.vector.tensor_copy(
                    st.qkT_t[m][:, s4 * 512:(s4 + 1) * 512], ps[:])

        def emit_v(pair):
            for si in range(NSC):
                ps = v_ps.tile([128, 128], f32, name="vps", tag="vps")
                for k in range(KC):
                    nc.tensor.matmul(
                        ps[:],
                        st.xt_t[k][:, si * 128:(si + 1) * 128],
                        st.wv_t[k][:, pair * 128:(pair + 1) * 128],
                        start=(k == 0),
                        stop=(k == KC - 1 and not has_bv),
                    )
                if has_bv:
                    nc.tensor.matmul(
                        ps[:], st.ones_row[0:1, 0:128],
                        st.bv_sb[0:1, pair * 128:(pair + 1) * 128],
                        start=False, stop=True,
                    )
                t = st.v_sb[pair][si]
                nc.scalar.copy(t[:, 0:64], ps[:, 0:64])
                nc.scalar.copy(t[:, 65:129], ps[:, 64:128])

        # pair 1's v projection is NOT emitted here: it runs inside the
        # attention phase as PE filler chains (emit_v_chain)
        emit_qk(0)
        emit_qk(1)
        emit_v(0)
        emit_qk(2)
        emit_qk(3)


def _phase_attn_out(ctx, tc, nc, d, st, has_bv):
    from concourse import mybir
    bf = mybir.dt.bfloat16
    f32 = mybir.dt.float32
    EXP = mybir.ActivationFunctionType.Exp

    with ExitStack() as cctx:
        # PSUM budget (8 banks): sc ring 3 x 2 banks + pv 2 x 1 bank. The
        # out-projection borrows sc-ring slots instead of its own pool; the
        # deep sc ring is what gives the PE enough runway to keep the HAM
        # clock-gate at 2.4GHz.
        sc_ps = cctx.enter_context(tc.tile_pool(name="sc_ps", bufs=3, space="PSUM"))
        pv_ps = cctx.enter_context(tc.tile_pool(name="pv_ps", bufs=2, space="PSUM"))
        rb_pool = cctx.enter_context(tc.tile_pool(name="erb", bufs=8))
        esc_pool = cctx.enter_context(tc.tile_pool(name="esc", bufs=5))
        pr_pool = cctx.enter_context(tc.tile_pool(name="prob", bufs=5))
        pvf_pool = cctx.enter_context(tc.tile_pool(name="pvf", bufs=4))
        pk_pool = cctx.enter_context(tc.tile_pool(name="pk", bufs=3))
        bc_pool = cctx.enter_context(tc.tile_pool(name="bc", bufs=3))
        dram_pool = cctx.enter_context(tc.tile_pool(name="recd", bufs=6, space="DRAM"))
        osb_pool = cctx.enter_context(tc.tile_pool(name="osb", bufs=4))

        out_q = deque()  # lagged out-projection chains (si, e2)
        v_q = deque()    # pair-1 v-projection chains (si,) as attn fillers

        def emit_out_chain():
            si, e2 = out_q.popleft()
            ps = sc_ps.tile([128, 2, 512], f32, name="sc", tag="sc")
            for p in range(2):
                nc.tensor.matmul(
                    ps[:, 0, :],
                    st.attnT_t[p][:, si * 128:(si + 1) * 128],
                    st.wo_t[p][:, e2 * 512:(e2 + 1) * 512],
                    start=(p == 0), stop=(p == 1),
                )
            osb = osb_pool.tile([128, 512], bf, name="osb", tag="osb")
            if (si + e2) % 2:
                nc.scalar.copy(osb[:], ps[:, 0, :])
            else:
                nc.vector.tensor_copy(osb[:], ps[:, 0, :])
            nc.sync.dma_start(
                d.out[si * 128:(si + 1) * 128, e2 * 512:(e2 + 1) * 512],
                osb[:])

        def emit_v_chain():
            si = v_q.popleft()
            ps = sc_ps.tile([128, 2, 512], f32, name="sc", tag="sc")
            vps = ps[:, 0, 0:128]
            for k in range(KC):
                nc.tensor.matmul(
                    vps,
                    st.xt_t[k][:, si * 128:(si + 1) * 128],
                    st.wv_t[k][:, 128:256],
                    start=(k == 0),
                    stop=(k == KC - 1 and not has_bv),
                )
            if has_bv:
                nc.tensor.matmul(
                    vps, st.ones_row[0:1, 0:128], st.bv_sb[0:1, 128:256],
                    start=False, stop=True,
                )
            t = st.v_sb[1][si]
            nc.scalar.copy(t[:, 0:64], vps[:, 0:64])
            nc.vector.tensor_copy(t[:, 65:129], vps[:, 64:128])

        def emit_filler():
            if v_q:
                emit_v_chain()
            elif out_q:
                emit_out_chain()

        def emit_attn_chunk(p, c):
            qT = st.qkT_t[2 * p]
            kT = st.qkT_t[2 * p + 1]
            nkj = 4 * (c + 1)
            pv_e = pv_ps.tile([65, 512], f32, name="pv", tag="pv")
            pv_o = pv_ps.tile([65, 512], f32, name="pv", tag="pv")
            pend = deque()

            def flush_pv():
                kjb, o, w, pr = pend.popleft()
                vt = st.v_sb[p][kjb]
                nc.tensor.matmul(
                    pv_e[0:65, o:o + w], vt[:, 0:65], pr[:, 0, o:o + w],
                    start=(kjb == 0), stop=(kjb == nkj - 1))
                nc.tensor.matmul(
                    pv_o[0:65, o:o + w], vt[:, 65:130], pr[:, 1, o:o + w],
                    start=(kjb == 0), stop=(kjb == nkj - 1))

            for kjb in range(nkj):
                o = max(0, kjb * 128 - c * 512)
                w = 512 - o
                q0 = c * 512 + o
                sc = sc_ps.tile([128, 2, 512], f32, name="sc", tag="sc")
                # both heads' score MMs adjacent: alternating PE row groups
                # let LDWEIGHTS overlap the in-flight matmul
                for h in range(2):
                    rows = slice(64 * h, 64 * h + 64)
                    nc.tensor.matmul(
                        sc[:, h, o:o + w],
                        kT[rows, kjb * 128:(kjb + 1) * 128],
                        qT[rows, q0:q0 + w],
                        start=True, stop=True,
                        tile_position=(64 * h, 0),
                    )
                esc = esc_pool.tile([128, 2, 512], bf, name="esc", tag="esc")
                nc.scalar.activation(esc[:, :, o:o + w], sc[:, :, o:o + w], EXP)
                rb = rb_pool.tile([128, 2, 512], bf, name="erb", tag="erb")
                nc.sync.dma_start(
                    rb[:, :, o:o + w],
                    d.erb[p, c, kjb * 128:(kjb + 1) * 128, :, o:o + w])
                pr = pr_pool.tile([128, 2, 512], bf, name="prob", tag="prob")
                # only the small partial blocks go to the (slow but idle)
                # GPSIMD; full blocks stay on the DVE's 2x bf16 path
                eng = nc.gpsimd if w <= 256 else nc.vector
                eng.tensor_mul(pr[:, :, o:o + w], esc[:, :, o:o + w],
                               rb[:, :, o:o + w])
                pend.append((kjb, o, w, pr))
                if len(pend) >= 3:
                    flush_pv()
                emit_filler()
            while pend:
                flush_pv()

            # evict both pv accumulators to SBUF immediately (bf16; the
            # rounding is relative so it cancels in the normalization) so the
            # 2-deep pv ring never stalls the next chunk's matmuls
            pvf_e = pvf_pool.tile([65, 512], bf, name="pvf", tag="pvf")
            pvf_o = pvf_pool.tile([65, 512], bf, name="pvf", tag="pvf")
            nc.scalar.copy(pvf_e[:], pv_e[:])
            nc.vector.tensor_copy(pvf_o[:], pv_o[:])

            # denominators (pvf row 64): pack 2x[1,512] into [64,16] via an
            # SBUF->SBUF DMA so the cast+reciprocal run 64-partition-parallel
            pk_b = pk_pool.tile([64, 16], bf, name="pkb", tag="pkb")
            nc.sync.dma_start(pk_b[0:32, :], pvf_e[64:65, :])
            nc.sync.dma_start(pk_b[32:64, :], pvf_o[64:65, :])
            pk_f = pk_pool.tile([64, 16], f32, name="pkf", tag="pkf")
            nc.vector.tensor_copy(pk_f[:], pk_b[:])
            rec = pk_pool.tile([64, 16], f32, name="rec", tag="rec")
            nc.vector.reciprocal_approx_fast(out=rec[:], in_=pk_f[:])
            # bf16 from here on: the norm multiply then runs all-2-byte
            # operands at the DVE's 2x rate
            recb = pk_pool.tile([64, 16], bf, name="recb", tag="recb")
            nc.vector.tensor_copy(recb[:], rec[:])
            dbc = dram_pool.tile([2, 512], bf, name="recd", tag="recd")
            nc.sync.dma_start(dbc[:], recb[:])
            # both halves at base partition 0: SBUF/SBUF tensor_tensor inputs
            # must share their base partition
            bc = bc_pool.tile([64, 1024], bf, name="bc", tag="bc")
            nc.sync.dma_start(bc[:, 0:512],
                              dbc[0:1, :].partition_broadcast(64))
            nc.sync.dma_start(bc[:, 512:1024],
                              dbc[1:2, :].partition_broadcast(64))
            cs = c * 512
            nc.vector.tensor_mul(
                st.attnT_t[p][0:64, cs:cs + 512], pvf_e[0:64, :],
                bc[:, 0:512])
            nc.vector.tensor_mul(
                st.attnT_t[p][64:128, cs:cs + 512], pvf_o[0:64, :],
                bc[:, 512:1024])

        # big chunks first: out chains for group c interleave into the next
        # (smaller) group as PE filler, and the final group is the smallest.
        # pair 1's v projection fills the first group's iterations.
        v_q.extend(range(NSC))
        order = list(range(NCH - 1, -1, -1))
        for gi, c in enumerate(order):
            emit_attn_chunk(0, c)
            emit_attn_chunk(1, c)
            if gi > 0:
                cprev = order[gi - 1]
                for si in range(4 * cprev, 4 * cprev + 4):
                    out_q.append((si, 0))
                    out_q.append((si, 1))
        for si in range(0, 4):
            out_q.append((si, 0))
            out_q.append((si, 1))
        while v_q:
            emit_v_chain()
        while out_q:
            emit_out_chain()


_LDW_OPT_INSTALLED = False


def _enable_ldw_opt():
    """walrus ships with --enable-ldw-opt=false; flip it for this process
    (dedupes/hoists LDWEIGHTS). Gated by KERNEL_LDW_OPT=1."""
    global _LDW_OPT_INSTALLED
    if _LDW_OPT_INSTALLED:
        return
    _LDW_OPT_INSTALLED = True
    import os
    if os.environ.get("KERNEL_LDW_OPT", "0") != "1":
        return
    import concourse.bass_utils as bu
    orig = bu.run_command

    def patched(argv, **kwargs):
        argv = ["--enable-ldw-opt=true" if a == "--enable-ldw-opt=false" else a
                for a in argv]
        return orig(argv, **kwargs)

    bu.run_command = patched


def _build_program(has_bqk: bool, has_bv: bool):
    import concourse.tile as tile
    from concourse import bacc, mybir

    bf = mybir.dt.bfloat16
    f32 = mybir.dt.float32

    nc = bacc.Bacc("TRN2", target_bir_lowering=False, debug=False,
                   num_devices=NCORES)

    d = types.SimpleNamespace()
    d.xT = nc.dram_tensor("xT", [D, S], bf, kind="ExternalInput").ap()
    d.wqk = nc.dram_tensor("wqk", [D, 512], bf, kind="ExternalInput").ap()
    d.wv = nc.dram_tensor("wv", [D, 256], bf, kind="ExternalInput").ap()
    d.bqk = nc.dram_tensor("bqk", [4, 128], bf, kind="ExternalInput").ap()
    d.bv = nc.dram_tensor("bv", [1, 256], bf, kind="ExternalInput").ap()
    d.erb = nc.dram_tensor("erb", [2, NCH, S, 2, 512], bf,
                           kind="ExternalInput").ap()
    d.wo = nc.dram_tensor("wo", [2, 128, D], bf, kind="ExternalInput").ap()
    d.out = nc.dram_tensor("out", [S, D], bf, kind="ExternalOutput").ap()

    st = types.SimpleNamespace()
    with tile.TileContext(nc) as tc:
        with ExitStack() as ctx:
            _phase_load(ctx, tc, nc, d, has_bqk, has_bv, st)
            _phase_proj(ctx, tc, nc, has_bqk, has_bv, st)
            _phase_attn_out(ctx, tc, nc, d, st, has_bv)

    nc.compile()
    return nc


_PROGRAM_CACHE = {}


def _get_program(has_bqk, has_bv):
    key = (has_bqk, has_bv)
    if key not in _PROGRAM_CACHE:
        _PROGRAM_CACHE[key] = _build_program(has_bqk, has_bv)
    return _PROGRAM_CACHE[key]


_last_results = None  # BassKernelResults of the most recent run (for test.py)


def kernel(x, rel_bias, w_qkv, b_qkv, w_out, b_out, *, trace=False):
    global _last_results
    _install_ntff_hook()
    _enable_ldw_opt()
    from concourse.bass_utils import run_bass_kernel_spmd

    x = np.asarray(x, dtype=np.float32)
    rel_bias = np.asarray(rel_bias, dtype=np.float32)
    w_qkv = np.asarray(w_qkv, dtype=np.float32)
    b_qkv = np.asarray(b_qkv, dtype=np.float32)
    w_out = np.asarray(w_out, dtype=np.float32)
    b_out = np.asarray(b_out, dtype=np.float32)

    wq = w_qkv[:, 0:D]
    wk = w_qkv[:, D:2 * D]
    wv = w_qkv[:, 2 * D:3 * D]
    bq, bk, bv = b_qkv[0:D], b_qkv[D:2 * D], b_qkv[2 * D:3 * D]
    has_bqk = bool(np.any(bq)) or bool(np.any(bk))
    has_bv = bool(np.any(bv))

    nc = _get_program(has_bqk, has_bv)

    sc = 1.0 / math.sqrt(HD)  # folded into the q projection
    xT = [np.ascontiguousarray(x[b].T).astype(_BF16) for b in range(B)]
    tri = np.triu(np.ones((S, S), dtype=np.float32))  # [kj, qi]: qi >= kj

    in_maps = []
    for c in range(NCORES):
        b, hg = divmod(c, 4)
        hs = [4 * hg + i for i in range(HPC)]

        # wqk columns: [q_h0 | q_h1 | k_h0 | k_h1 | q_h2 | q_h3 | k_h2 | k_h3]
        cols = []
        bqk_rows = []
        for pair in range(2):
            h0, h1 = hs[2 * pair], hs[2 * pair + 1]
            cols += [wq[:, HD * h0:HD * (h0 + 1)] * sc,
                     wq[:, HD * h1:HD * (h1 + 1)] * sc]
            bqk_rows.append(np.concatenate(
                [bq[HD * h0:HD * (h0 + 1)], bq[HD * h1:HD * (h1 + 1)]]) * sc)
            cols += [wk[:, HD * h0:HD * (h0 + 1)],
                     wk[:, HD * h1:HD * (h1 + 1)]]
            bqk_rows.append(np.concatenate(
                [bk[HD * h0:HD * (h0 + 1)], bk[HD * h1:HD * (h1 + 1)]]))
        wqk_c = np.concatenate(cols, axis=1).astype(_BF16)
        bqk_c = np.stack(bqk_rows).astype(_BF16)

        wv_c = np.zeros((D, 256), dtype=np.float32)
        bv_c = np.zeros((1, 256), dtype=np.float32)
        for pair in range(2):
            he, ho = hs[2 * pair], hs[2 * pair + 1]
            wv_c[:, pair * 128:pair * 128 + 64] = wv[:, HD * he:HD * (he + 1)]
            wv_c[:, pair * 128 + 64:pair * 128 + 128] = \
                wv[:, HD * ho:HD * (ho + 1)]
            bv_c[0, pair * 128:pair * 128 + 64] = bv[HD * he:HD * (he + 1)]
            bv_c[0, pair * 128 + 64:pair * 128 + 128] = \
                bv[HD * ho:HD * (ho + 1)]

        # erb[pair, chunk, kj, h, qi_in_chunk] = exp(rel_bias^T) * causal
        erb_c = np.empty((2, NCH, S, 2, 512), dtype=_BF16)
        for pair in range(2):
            for i_h in range(2):
                head = hs[2 * pair + i_h]
                m = (np.exp(rel_bias[head].T) * tri).astype(_BF16)
                erb_c[pair, :, :, i_h, :] = \
                    m.reshape(S, NCH, 512).transpose(1, 0, 2)

        in_maps.append({
            "xT": xT[b],
            "wqk": wqk_c,
            "wv": wv_c.astype(_BF16),
            "bqk": bqk_c,
            "bv": bv_c.astype(_BF16),
            "erb": erb_c,
            "wo": np.ascontiguousarray(
                w_out[256 * hg:256 * (hg + 1)].reshape(2, 128, D)).astype(_BF16),
        })

    res = run_bass_kernel_spmd(nc, in_maps, list(range(NCORES)), trace=trace)
    _last_results = res

    out = np.zeros((B, S, D), dtype=np.float32)
    for c in range(NCORES):
        out[c // 4] += np.asarray(res.results[c]["out"], dtype=np.float32)
    out += b_out
    return out
